# revision 1
# baseline (speedup 1.0000x reference)
"""Multi-head attention (B=2, S=2048, D=1024, H=16, d_k=64) on 8 NeuronCores.

Sharding: 8 cores = 2 batches x 4 head-groups (4 heads each).
Core c handles batch b = c//4 and heads 4*(c%4) .. 4*(c%4)+4 (feature
slice of width F=256). Each core computes its partial output-projection
contribution [S, D]; the host sums the 4 head-group partials per batch
and adds b4 (the "all-reduce" of the row-sharded W4 projection).

Device dataflow works in a "transposed world" so every matmul operand
is in its natural PE layout (contraction on partitions), with zero
on-device transposes:
  qT = W1g @ x_q.T  [F, S]   (lhsT = W1g.T host-prepped, rhs = x_q.T host-prepped)
  kT = W2g @ x_k.T  [F, S]
  v  = x_v @ W3g.T  [S, F]   (lhsT = x_v.T, rhs = W3g.T; bias via K=1 ones matmul)
  scoresT_h = kT_h.T @ qT_h        [S_keys, S_q]   (K = d_k = 64; 2 heads packed
                                                    in PE row groups 0:64 / 64:128)
  attnT = exp(scoresT / 8)          ACT, PSUM->SBUF bf16, no max subtraction
                                    (scores ~ N(0,1); max|score/8| ~ 10 -> safe in f32)
  pv = [v_h | ones].T @ attnT      [65, S_q]; row 64 = softmax denominator
  outT_h = pv[0:64] * (1/pv[64])   per-query normalization (flash-style, post-PV)
  partial = outT.T @ W4g.T         [S, D]  (lhsT = outT, rhs = W4g.T host-prepped)

All matmuls bf16 with f32 PSUM accumulation (validated 0.6% L2 rel err).

Schedule: attention is processed per (head-pair, query-half) window; within
a window, scores MMs (kt) and PV MMs (kt-1) interleave per key tile so the
PE stream has no multi-us stalls (keeps the HAM clock gate warm) while ACT
exp paces the pipeline. PSUM: scores 2x2 banks + PV accumulators 2x2 banks;
projection / output-projection psum recycles the same tags.
"""

import numpy as np
import ml_dtypes

import concourse.bass as bass
import concourse.mybir as mybir
import concourse.tile as tile
from concourse import bacc
from concourse.bass_utils import run_bass_kernel_spmd

BF16 = ml_dtypes.bfloat16
F32 = mybir.dt.float32
BF = mybir.dt.bfloat16

B, S, D = 2, 2048, 1024
H_CORE = 4          # heads per core
DK = 64             # head dim
F = H_CORE * DK     # features per core = 256
P = 128             # partitions
KB = D // P         # k blocks in D contraction = 8
SM = S // P         # seq tiles of 128 = 16
QW = 1024           # query window width
NQW = S // QW       # query windows = 2
N_CORES = 8


DEBUG_TAPS = False
EXACT_RECIP = True


def _build_kernel():
    nc = bacc.Bacc(
        "TRN2",
        target_bir_lowering=False,
        debug=False,
        enable_asserts=False,
        num_devices=N_CORES,
    )

    xq = nc.dram_tensor("xq_t", [D, S], BF, kind="ExternalInput").ap()
    xk = nc.dram_tensor("xk_t", [D, S], BF, kind="ExternalInput").ap()
    xv = nc.dram_tensor("xv_t", [D, S], BF, kind="ExternalInput").ap()
    w1 = nc.dram_tensor("w1t", [D, F], BF, kind="ExternalInput").ap()
    w2 = nc.dram_tensor("w2t", [D, F], BF, kind="ExternalInput").ap()
    w3 = nc.dram_tensor("w3t", [D, F], BF, kind="ExternalInput").ap()
    w4 = nc.dram_tensor("w4t", [F, D], BF, kind="ExternalInput").ap()
    b1 = nc.dram_tensor("b1c", [P, F // P], F32, kind="ExternalInput").ap()
    b2 = nc.dram_tensor("b2c", [P, F // P], F32, kind="ExternalInput").ap()
    b3 = nc.dram_tensor("b3r", [1, F], BF, kind="ExternalInput").ap()
    out = nc.dram_tensor("out", [S, D], F32, kind="ExternalOutput").ap()
    taps = None
    if DEBUG_TAPS:
        taps = {
            "dbg_qT0": nc.dram_tensor("dbg_qT0", [P, S], BF, kind="ExternalOutput").ap(),
            "dbg_kT0": nc.dram_tensor("dbg_kT0", [P, S], BF, kind="ExternalOutput").ap(),
            "dbg_v0": nc.dram_tensor("dbg_v0", [P, 260], BF, kind="ExternalOutput").ap(),
            "dbg_outT0": nc.dram_tensor("dbg_outT0", [P, S], BF, kind="ExternalOutput").ap(),
        }

    with tile.TileContext(nc) as tc:
        _body(tc, xq, xk, xv, w1, w2, w3, w4, b1, b2, b3, out, taps)

    nc.compile()
    return nc


def _body(tc, xq, xk, xv, w1, w2, w3, w4, b1, b2, b3, out, taps=None):
    nc = tc.nc
    MF = F // P  # m tiles for the F=256 feature dim = 2

    with (
        tc.tile_pool(name="wpool", bufs=1) as wpool,
        tc.tile_pool(name="xt", bufs=20) as xt_pool,
        tc.tile_pool(name="persist", bufs=1) as persist,
        tc.tile_pool(name="attn", bufs=6) as attn_pool,
        tc.tile_pool(name="small", bufs=4) as small,
        tc.tile_pool(name="stage", bufs=2) as stage,
        tc.tile_pool(name="psum", bufs=1, space="PSUM") as psum,
    ):
        # ---- weights / constants to SBUF ----
        w1_sb = [wpool.tile([P, F], BF, name=f"w1_{k}", tag=f"w1_{k}") for k in range(KB)]
        w2_sb = [wpool.tile([P, F], BF, name=f"w2_{k}", tag=f"w2_{k}") for k in range(KB)]
        w3_sb = [wpool.tile([P, F], BF, name=f"w3_{k}", tag=f"w3_{k}") for k in range(KB)]
        w4_sb = [wpool.tile([P, D], BF, name=f"w4_{k}", tag=f"w4_{k}") for k in range(MF)]
        for k in range(KB):
            nc.sync.dma_start(w1_sb[k][:], w1[k * P:(k + 1) * P, :])
            nc.sync.dma_start(w2_sb[k][:], w2[k * P:(k + 1) * P, :])
            nc.sync.dma_start(w3_sb[k][:], w3[k * P:(k + 1) * P, :])
        for k in range(MF):
            nc.sync.dma_start(w4_sb[k][:], w4[k * P:(k + 1) * P, :])
        b1_sb = wpool.tile([P, MF], F32, name="b1_sb", tag="b1_sb")
        b2_sb = wpool.tile([P, MF], F32, name="b2_sb", tag="b2_sb")
        b3_sb = wpool.tile([1, F], BF, name="b3_sb", tag="b3_sb")
        nc.sync.dma_start(b1_sb[:], b1[:])
        nc.sync.dma_start(b2_sb[:], b2[:])
        nc.sync.dma_start(b3_sb[:], b3[:])
        ones_row = wpool.tile([1, P], BF, name="ones_row", tag="ones_row")
        nc.vector.memset(ones_row[:], 1.0)

        # persistent activations
        qT = [persist.tile([P, S], BF, name=f"qT_{m}", tag=f"qT_{m}") for m in range(MF)]
        kT = [persist.tile([P, S], BF, name=f"kT_{m}", tag=f"kT_{m}") for m in range(MF)]
        # v with interleaved ones columns: per head h, cols 65h..65h+63 = v_h,
        # col 65h+64 = 1.0 (softmax denominator trick)
        VW = H_CORE * (DK + 1)  # 260
        v_sb = [persist.tile([P, VW], BF, name=f"v_{s}", tag=f"v_{s}") for s in range(SM)]
        for s in range(SM):
            for h in range(H_CORE):
                nc.vector.memset(v_sb[s][:, h * (DK + 1) + DK: h * (DK + 1) + DK + 1], 1.0)
        outT = [persist.tile([P, S], BF, name=f"outT_{m}", tag=f"outT_{m}") for m in range(MF)]

        # ---- q / k projections: qT[m][:, qw] = sum_k W1t[k][:,m].T @ xq[k][:,qw] ----
        for name, x_dram, w_sb, b_sb, dst in (
            ("q", xq, w1_sb, b1_sb, qT),
            ("k", xk, w2_sb, b2_sb, kT),
        ):
            x_sb = []
            for k in range(KB):
                t = xt_pool.tile([P, S], BF, name=f"x{name}_{k}", tag="xt")
                nc.sync.dma_start(t[:], x_dram[k * P:(k + 1) * P, :])
                x_sb.append(t)
            for m in range(MF):
                for qw in range(NQW):
                    ps = psum.tile([P, QW], F32, name=f"pp_{name}_{m}_{qw}", tag="sc", bufs=2)
                    for k in range(KB):
                        for half in range(2):
                            nc.tensor.matmul(
                                ps[:, half * 512:(half + 1) * 512],
                                w_sb[k][:, m * P:(m + 1) * P],
                                x_sb[k][:, qw * QW + half * 512: qw * QW + (half + 1) * 512],
                                start=(k == 0),
                                stop=(k == KB - 1),
                            )
                    nc.vector.tensor_scalar_add(
                        dst[m][:, qw * QW:(qw + 1) * QW], ps[:], b_sb[:, m:m + 1]
                    )

        # ---- v projection (natural layout): v[s] = xv[:, s].T @ W3t + b3 ----
        x_sb = []
        for k in range(KB):
            t = xt_pool.tile([P, S], BF, name=f"xv_{k}", tag="xt")
            nc.sync.dma_start(t[:], xv[k * P:(k + 1) * P, :])
            x_sb.append(t)
        for s in range(SM):
            ps = psum.tile([P, F], F32, name=f"pv_{s}", tag="pv", bufs=2)
            for k in range(KB):
                nc.tensor.matmul(
                    ps[:],
                    x_sb[k][:, s * P:(s + 1) * P],
                    w3_sb[k][:],
                    start=(k == 0),
                    stop=False,
                )
            # bias: += ones.T @ b3  (K=1)
            nc.tensor.matmul(ps[:], ones_row[:], b3_sb[:], start=False, stop=True)
            for h in range(H_CORE):
                nc.vector.tensor_copy(
                    v_sb[s][:, h * (DK + 1): h * (DK + 1) + DK],
                    ps[:, h * DK:(h + 1) * DK],
                )

        # ---- output projection step generator (used as PE filler + tail) ----
        def gen_w4(qts):
            for qt in qts:
                ps = psum.tile([P, D], F32, name=f"po_{qt}", tag="sc", bufs=2)
                for oc in range(D // 512):
                    for m in range(MF):
                        nc.tensor.matmul(
                            ps[:, oc * 512:(oc + 1) * 512],
                            outT[m][:, qt * P:(qt + 1) * P],
                            w4_sb[m][:, oc * 512:(oc + 1) * 512],
                            start=(m == 0),
                            stop=(m == MF - 1),
                        )
                    yield
                ob = stage.tile([P, D], F32, name=f"ob_{qt}", tag="ob")
                nc.vector.tensor_copy(ob[:], ps[:])
                nc.sync.dma_start(out[qt * P:(qt + 1) * P, :], ob[:])
                yield

        # ---- attention, per head-pair hp (heads 2hp, 2hp+1 live in qT/kT tile hp),
        #      per query window qw; scores(kt) and PV(kt-1) interleaved per key tile.
        #      filler: (start_slot, steps_per_slot, generator) for PE idle slots ----
        def window(hp, qw, filler=None):
            qsl = slice(qw * QW, (qw + 1) * QW)
            attn_t = [[None] * SM for _ in range(2)]
            pv_ps = [
                psum.tile([P, QW], F32, name=f"pvps_{hp}_{qw}_{h2}", tag="pv", bufs=2)
                for h2 in range(2)
            ]

            def emit_scores(kt):
                for h2 in range(2):
                    rsl = slice(h2 * DK, (h2 + 1) * DK)
                    ps = psum.tile([P, QW], F32, name=f"sc_{hp}_{qw}_{kt}_{h2}",
                                   tag="sc", bufs=2)
                    for half in range(2):
                        nc.tensor.matmul(
                            ps[:, half * 512:(half + 1) * 512],
                            kT[hp][rsl, kt * P:(kt + 1) * P],
                            qT[hp][rsl, qw * QW + half * 512: qw * QW + (half + 1) * 512],
                            start=True,
                            stop=True,
                        )
                    at = attn_pool.tile([P, QW], BF, name=f"at_{hp}_{qw}_{kt}_{h2}",
                                        tag="attnT", bufs=6)
                    nc.scalar.activation(
                        at[:], ps[:], mybir.ActivationFunctionType.Exp,
                        scale=1.0 / np.sqrt(DK),
                    )
                    attn_t[h2][kt] = at

            def emit_pv(kt):
                for h2 in range(2):
                    h = hp * 2 + h2
                    vsl = slice(h * (DK + 1), h * (DK + 1) + DK + 1)
                    for half in range(2):
                        nc.tensor.matmul(
                            pv_ps[h2][0:DK + 1, half * 512:(half + 1) * 512],
                            v_sb[kt][:, vsl],
                            attn_t[h2][kt][:, half * 512:(half + 1) * 512],
                            start=(kt == 0),
                            stop=(kt == SM - 1),
                        )

            emit_scores(0)
            for kt in range(1, SM):
                emit_scores(kt)
                emit_pv(kt - 1)
                if filler is not None and kt >= filler[0]:
                    for _ in range(filler[1]):
                        next(filler[2], None)
            emit_pv(SM - 1)

            # fast PSUM release: copy denominators + raw (unnormalized) outT,
            # then normalize off the critical path.
            dens, raws = [], []
            for h2 in range(2):
                den = small.tile([1, QW], F32, name=f"den_{hp}_{qw}_{h2}", tag="den", bufs=3)
                nc.vector.tensor_copy(den[:], pv_ps[h2][DK:DK + 1, :])
                dens.append(den)
            for h2 in range(2):
                raw = small.tile([DK, QW], BF, name=f"raw_{hp}_{qw}_{h2}", tag="raw", bufs=3)
                nc.vector.tensor_copy(raw[:], pv_ps[h2][0:DK, :])
                raws.append(raw)
            for h2 in range(2):
                rec = small.tile([1, QW], F32, name=f"rec_{hp}_{qw}_{h2}", tag="rec", bufs=3)
                nc.vector.reciprocal(rec[:], dens[h2][:])
                bc = small.tile([DK, QW], F32, name=f"bc_{hp}_{qw}_{h2}", tag="bc", bufs=2)
                nc.gpsimd.partition_broadcast(bc[:], rec[:])
                nc.vector.tensor_mul(
                    outT[hp][h2 * DK:(h2 + 1) * DK, qsl], raws[h2][:], bc[:]
                )

        window(0, 0)
        window(0, 1)
        window(1, 0)
        window(1, 1)
        for _ in gen_w4(range(SM)):
            pass

        if taps is not None:
            nc.sync.dma_start(taps["dbg_qT0"][:], qT[0][:])
            nc.sync.dma_start(taps["dbg_kT0"][:], kT[0][:])
            nc.sync.dma_start(taps["dbg_v0"][:], v_sb[0][:])
            nc.sync.dma_start(taps["dbg_outT0"][:], outT[0][:])


_NC_CACHE = None


def _get_nc():
    global _NC_CACHE
    if _NC_CACHE is None:
        _NC_CACHE = _build_kernel()
    return _NC_CACHE


def _make_in_maps(query, key, value, W1, b1, W2, b2, W3, b3, W4, b4):
    in_maps = []
    for c in range(N_CORES):
        b, g = divmod(c, 4)
        gs = slice(g * F, (g + 1) * F)
        in_maps.append({
            "xq_t": np.ascontiguousarray(query[b].T).astype(BF16),
            "xk_t": np.ascontiguousarray(key[b].T).astype(BF16),
            "xv_t": np.ascontiguousarray(value[b].T).astype(BF16),
            "w1t": np.ascontiguousarray(W1[gs, :].T).astype(BF16),
            "w2t": np.ascontiguousarray(W2[gs, :].T).astype(BF16),
            "w3t": np.ascontiguousarray(W3[gs, :].T).astype(BF16),
            "w4t": np.ascontiguousarray(W4[:, gs].T).astype(BF16),
            "b1c": np.ascontiguousarray(b1[gs].reshape(F // P, P).T).astype(np.float32),
            "b2c": np.ascontiguousarray(b2[gs].reshape(F // P, P).T).astype(np.float32),
            "b3r": b3[gs].reshape(1, F).astype(BF16),
        })
    return in_maps


def kernel(query, key, value, W1, b1, W2, b2, W3, b3, W4, b4, _trace=False, _tmpdir=None):
    args = [np.asarray(a) for a in (query, key, value, W1, b1, W2, b2, W3, b3, W4, b4)]
    nc = _get_nc()
    in_maps = _make_in_maps(*args)
    res = run_bass_kernel_spmd(
        nc, in_maps, core_ids=list(range(N_CORES)),
        trace=_trace, tmpdir=_tmpdir,
    )
    b4_f = args[10].astype(np.float32)
    full = np.zeros((B, S, D), np.float32)
    for c in range(N_CORES):
        full[c // 4] += res.results[c]["out"]
    full += b4_f[None, None, :]
    kernel.last_results = res
    return full



# revision 4
# speedup vs baseline: 1.0147x; 1.0147x over previous
"""Multi-head attention (B=2, S=2048, D=1024, H=16, d_k=64) on 8 NeuronCores.

Sharding: 8 cores = 2 batches x 4 head-groups (4 heads each).
Core c handles batch b = c//4 and heads 4*(c%4) .. 4*(c%4)+4 (feature
slice of width F=256). Each core computes its partial output-projection
contribution [S, D]; the host sums the 4 head-group partials per batch
and adds b4 (the "all-reduce" of the row-sharded W4 projection).

Device dataflow works in a "transposed world" so every matmul operand
is in its natural PE layout (contraction on partitions), with zero
on-device transposes:
  qT = W1g @ x_q.T  [F, S]
  kT = W2g @ x_k.T  [F, S]
  v  = x_v @ W3i    [S, 260]  (260 = 4 heads x (64 v cols + 1 ones col);
                               the ones col comes from the bias matmul
                               with b3i[h*65+64] = 1, W3i zero there)
  scoresT_h = kT_h.T @ qT_h   [S_keys, S_q]  (K = d_k = 64)
  attnT = exp(scoresT / 8)    ACT, PSUM->SBUF bf16, no max subtraction
  pv = v_ext.T @ attnT        [65, S_q]; row 64 = softmax denominator
  outT_h = pv[0:64] * (1/pv[64])   (reciprocal_approx_fast + gpsimd
                                    partition_broadcast + one DVE mul)
  partial = outT.T @ W4g.T    [S, D]

All matmuls bf16 with f32 PSUM accumulation.

Schedule (built to keep the PE stream dense so the HAM clock gate stays
at 8/8, and to hide everything under the ACT exp roofline):
  - prioritized chunked DMA: w1 + xq first (512-col chunks) so the
    first projection matmuls issue ~5us in
  - upfront PE work: only the m=0 half of the q/k projections (enough
    for head-pair 0's windows)
  - window (0,0): scores/exp/PV with the v-projection interleaved as
    PE filler (one s-tile per kt step, sharing the "sc" psum tag)
  - window (0,1): m=1 q/k projection groups as filler
  - window (1,0): no filler
  - window (1,1): W4 output projection for qw=0 as filler
  - tail: W4 for qw=1
  PV lags scores by 2 key tiles so PV never waits on the exp of the
  same step (ACT latency hidden at any clock).  PSUM: scores 2x2 banks
  (tag sc, shared by all filler psum) + PV accumulators 2x2 banks.
"""

import numpy as np
import ml_dtypes

import concourse.bass as bass
import concourse.mybir as mybir
import concourse.tile as tile
from concourse import bacc
from concourse.bass_utils import run_bass_kernel_spmd

BF16 = ml_dtypes.bfloat16
F32 = mybir.dt.float32
BF = mybir.dt.bfloat16

B, S, D = 2, 2048, 1024
H_CORE = 4          # heads per core
DK = 64             # head dim
F = H_CORE * DK     # features per core = 256
P = 128             # partitions
KB = D // P         # k blocks in D contraction = 8
SM = S // P         # seq tiles of 128 = 16
QW = 1024           # query window width
NQW = S // QW       # query windows = 2
VW = H_CORE * (DK + 1)  # 260: v with interleaved ones columns
N_CORES = 8


def _build_kernel():
    nc = bacc.Bacc(
        "TRN2",
        target_bir_lowering=False,
        debug=False,
        enable_asserts=False,
        num_devices=N_CORES,
    )

    xq = nc.dram_tensor("xq_t", [D, S], BF, kind="ExternalInput").ap()
    xk = nc.dram_tensor("xk_t", [D, S], BF, kind="ExternalInput").ap()
    xv = nc.dram_tensor("xv_t", [D, S], BF, kind="ExternalInput").ap()
    w1 = nc.dram_tensor("w1t", [D, F], BF, kind="ExternalInput").ap()
    w2 = nc.dram_tensor("w2t", [D, F], BF, kind="ExternalInput").ap()
    w3 = nc.dram_tensor("w3i", [D, VW], BF, kind="ExternalInput").ap()
    w4 = nc.dram_tensor("w4t", [F, D], BF, kind="ExternalInput").ap()
    b1 = nc.dram_tensor("b1c", [P, F // P], F32, kind="ExternalInput").ap()
    b2 = nc.dram_tensor("b2c", [P, F // P], F32, kind="ExternalInput").ap()
    b3 = nc.dram_tensor("b3i", [1, VW], BF, kind="ExternalInput").ap()
    out = nc.dram_tensor("out", [S, D], F32, kind="ExternalOutput").ap()

    with tile.TileContext(nc) as tc:
        _body(tc, xq, xk, xv, w1, w2, w3, w4, b1, b2, b3, out)

    nc.compile()
    return nc


def _body(tc, xq, xk, xv, w1, w2, w3, w4, b1, b2, b3, out):
    nc = tc.nc
    MF = F // P  # m tiles for the F=256 feature dim = 2

    with (
        tc.tile_pool(name="wpool", bufs=1) as wpool,
        tc.tile_pool(name="xt", bufs=24) as xt_pool,
        tc.tile_pool(name="persist", bufs=1) as persist,
        tc.tile_pool(name="attn", bufs=6) as attn_pool,
        tc.tile_pool(name="small", bufs=3) as small,
        tc.tile_pool(name="stage", bufs=2) as stage,
        tc.tile_pool(name="psum", bufs=1, space="PSUM") as psum,
    ):
        # ---- SBUF tiles; DMA issued in priority order ----
        w1_sb = [wpool.tile([P, F], BF, name=f"w1_{k}", tag=f"w1_{k}") for k in range(KB)]
        w2_sb = [wpool.tile([P, F], BF, name=f"w2_{k}", tag=f"w2_{k}") for k in range(KB)]
        w3_sb = [wpool.tile([P, VW], BF, name=f"w3_{k}", tag=f"w3_{k}") for k in range(KB)]
        w4_sb = [wpool.tile([P, D], BF, name=f"w4_{k}", tag=f"w4_{k}") for k in range(MF)]
        xq_sb = [xt_pool.tile([P, S], BF, name=f"xq_{k}", tag="xt") for k in range(KB)]
        xk_sb = [xt_pool.tile([P, S], BF, name=f"xk_{k}", tag="xt") for k in range(KB)]
        xv_sb = [xt_pool.tile([P, S], BF, name=f"xv_{k}", tag="xt") for k in range(KB)]
        b1_sb = wpool.tile([P, MF], F32, name="b1_sb", tag="b1_sb")
        b2_sb = wpool.tile([P, MF], F32, name="b2_sb", tag="b2_sb")
        b3_sb = wpool.tile([1, VW], BF, name="b3_sb", tag="b3_sb")
        ones_row = wpool.tile([1, P], BF, name="ones_row", tag="ones_row")
        nc.vector.memset(ones_row[:], 1.0)

        # priority: w1, xq (512-col chunks), w2, xk, w3, xv, w4, biases
        for k in range(KB):
            nc.sync.dma_start(w1_sb[k][:], w1[k * P:(k + 1) * P, :])
        for c in range(4):
            csl = slice(c * 512, (c + 1) * 512)
            for k in range(KB):
                nc.sync.dma_start(xq_sb[k][:, csl], xq[k * P:(k + 1) * P, csl])
        for k in range(KB):
            nc.sync.dma_start(w2_sb[k][:], w2[k * P:(k + 1) * P, :])
        for c in range(2):
            csl = slice(c * 1024, (c + 1) * 1024)
            for k in range(KB):
                nc.sync.dma_start(xk_sb[k][:, csl], xk[k * P:(k + 1) * P, csl])
        for k in range(KB):
            nc.sync.dma_start(w3_sb[k][:], w3[k * P:(k + 1) * P, :])
        for c in range(2):
            csl = slice(c * 1024, (c + 1) * 1024)
            for k in range(KB):
                nc.sync.dma_start(xv_sb[k][:, csl], xv[k * P:(k + 1) * P, csl])
        for k in range(MF):
            nc.sync.dma_start(w4_sb[k][:], w4[k * P:(k + 1) * P, :])
        nc.sync.dma_start(b1_sb[:], b1[:])
        nc.sync.dma_start(b2_sb[:], b2[:])
        nc.sync.dma_start(b3_sb[:], b3[:])

        # persistent activations
        qT = [persist.tile([P, S], BF, name=f"qT_{m}", tag=f"qT_{m}") for m in range(MF)]
        kT = [persist.tile([P, S], BF, name=f"kT_{m}", tag=f"kT_{m}") for m in range(MF)]
        v_sb = [persist.tile([P, VW], BF, name=f"v_{s}", tag=f"v_{s}") for s in range(SM)]
        outT = [persist.tile([P, S], BF, name=f"outT_{m}", tag=f"outT_{m}") for m in range(MF)]

        # ---- one q/k projection group: dst[m][:, 512-col slice] ----
        def proj_qk(name, x_sb, w_sb, b_sb, dst, m, h):
            csl = slice(h * 512, (h + 1) * 512)
            ps = psum.tile([P, 512], F32, name=f"pp_{name}_{m}_{h}", tag="sc", bufs=2)
            for k in range(KB):
                nc.tensor.matmul(
                    ps[:],
                    w_sb[k][:, m * P:(m + 1) * P],
                    x_sb[k][:, csl],
                    start=(k == 0),
                    stop=(k == KB - 1),
                )
            nc.vector.tensor_scalar_add(dst[m][:, csl], ps[:], b_sb[:, m:m + 1])

        # upfront: m=0 projections of q and k (feeds head-pair 0 windows)
        for h in range(4):
            proj_qk("q", xq_sb, w1_sb, b1_sb, qT, 0, h)
        for h in range(4):
            proj_qk("k", xk_sb, w2_sb, b2_sb, kT, 0, h)

        # ---- filler generators (consumed inside windows at PE idle slots) ----
        def gen_vproj():
            for s in range(SM):
                ps = psum.tile([P, VW], F32, name=f"pv_{s}", tag="sc", bufs=2)
                for k in range(KB):
                    nc.tensor.matmul(
                        ps[:],
                        xv_sb[k][:, s * P:(s + 1) * P],
                        w3_sb[k][:],
                        start=(k == 0),
                        stop=False,
                    )
                nc.tensor.matmul(ps[:], ones_row[:], b3_sb[:], start=False, stop=True)
                nc.vector.tensor_copy(v_sb[s][:], ps[:])
                yield

        def gen_m1proj():
            for name, x_sb, w_sb, b_sb, dst in (
                ("q1", xq_sb, w1_sb, b1_sb, qT),
                ("k1", xk_sb, w2_sb, b2_sb, kT),
            ):
                for h in range(4):
                    proj_qk(name, x_sb, w_sb, b_sb, dst, 1, h)
                    yield

        def gen_w4(qts):
            for qt in qts:
                ps = psum.tile([P, D], F32, name=f"po_{qt}", tag="sc", bufs=2)
                for oc in range(D // 512):
                    for m in range(MF):
                        nc.tensor.matmul(
                            ps[:, oc * 512:(oc + 1) * 512],
                            outT[m][:, qt * P:(qt + 1) * P],
                            w4_sb[m][:, oc * 512:(oc + 1) * 512],
                            start=(m == 0),
                            stop=(m == MF - 1),
                        )
                ob = stage.tile([P, D], F32, name=f"ob_{qt}", tag="ob")
                nc.vector.tensor_copy(ob[:], ps[:])
                nc.sync.dma_start(out[qt * P:(qt + 1) * P, :], ob[:])
                yield

        # ---- attention window: head-pair hp, query window qw.
        #      scores(kt) / PV(kt-2) interleave; filler consumed each step. ----
        def window(hp, qw, filler=None, fill_every=1):
            qsl = slice(qw * QW, (qw + 1) * QW)
            attn_t = [[None] * SM for _ in range(2)]
            pv_ps = [
                psum.tile([P, QW], F32, name=f"pvps_{hp}_{qw}_{h2}", tag="pv", bufs=2)
                for h2 in range(2)
            ]

            def emit_scores(kt):
                for h2 in range(2):
                    rsl = slice(h2 * DK, (h2 + 1) * DK)
                    ps = psum.tile([P, QW], F32, name=f"sc_{hp}_{qw}_{kt}_{h2}",
                                   tag="sc", bufs=2)
                    for half in range(2):
                        nc.tensor.matmul(
                            ps[:, half * 512:(half + 1) * 512],
                            kT[hp][rsl, kt * P:(kt + 1) * P],
                            qT[hp][rsl, qw * QW + half * 512: qw * QW + (half + 1) * 512],
                            start=True,
                            stop=True,
                        )
                    at = attn_pool.tile([P, QW], BF, name=f"at_{hp}_{qw}_{kt}_{h2}",
                                        tag="attnT", bufs=6)
                    nc.scalar.activation(
                        at[:], ps[:], mybir.ActivationFunctionType.Exp,
                        scale=1.0 / np.sqrt(DK),
                    )
                    attn_t[h2][kt] = at

            def emit_pv(kt):
                for h2 in range(2):
                    h = hp * 2 + h2
                    vsl = slice(h * (DK + 1), h * (DK + 1) + DK + 1)
                    for half in range(2):
                        nc.tensor.matmul(
                            pv_ps[h2][0:DK + 1, half * 512:(half + 1) * 512],
                            v_sb[kt][:, vsl],
                            attn_t[h2][kt][:, half * 512:(half + 1) * 512],
                            start=(kt == 0),
                            stop=(kt == SM - 1),
                        )

            for kt in range(SM):
                emit_scores(kt)
                if kt >= 2:
                    emit_pv(kt - 2)
                if filler is not None and kt % fill_every == fill_every - 1:
                    next(filler, None)
            emit_pv(SM - 2)
            emit_pv(SM - 1)

            # normalization: rec = 1/den straight off the PSUM denominator
            # row, broadcast on gpsimd, single DVE multiply into outT.
            for h2 in range(2):
                den = small.tile([1, QW], F32, name=f"den_{hp}_{qw}_{h2}", tag="den", bufs=3)
                nc.vector.tensor_copy(den[:], pv_ps[h2][DK:DK + 1, :])
                rec = small.tile([1, QW], F32, name=f"rec_{hp}_{qw}_{h2}", tag="rec", bufs=3)
                nc.vector.reciprocal_approx_fast(rec[:], den[:])
                bc = small.tile([DK, QW], F32, name=f"bc_{hp}_{qw}_{h2}", tag="bc", bufs=2)
                nc.gpsimd.partition_broadcast(bc[:], rec[:])
                nc.vector.tensor_mul(
                    outT[hp][h2 * DK:(h2 + 1) * DK, qsl], pv_ps[h2][0:DK, :], bc[:]
                )

        window(0, 0, filler=gen_vproj(), fill_every=1)
        window(0, 1, filler=gen_m1proj(), fill_every=2)
        window(1, 0)
        window(1, 1, filler=gen_w4(range(SM // 2)), fill_every=2)
        for _ in gen_w4(range(SM // 2, SM)):
            pass


_NC_CACHE = None


def _get_nc():
    global _NC_CACHE
    if _NC_CACHE is None:
        _NC_CACHE = _build_kernel()
    return _NC_CACHE


def _make_in_maps(query, key, value, W1, b1, W2, b2, W3, b3, W4, b4):
    in_maps = []
    for c in range(N_CORES):
        b, g = divmod(c, 4)
        gs = slice(g * F, (g + 1) * F)
        w3g = W3[gs, :].T.astype(np.float32)          # [D, F]
        w3i = np.zeros((D, VW), np.float32)
        b3g = b3[gs].astype(np.float32)
        b3i = np.zeros((VW,), np.float32)
        for h in range(H_CORE):
            w3i[:, h * (DK + 1): h * (DK + 1) + DK] = w3g[:, h * DK:(h + 1) * DK]
            b3i[h * (DK + 1): h * (DK + 1) + DK] = b3g[h * DK:(h + 1) * DK]
            b3i[h * (DK + 1) + DK] = 1.0
        in_maps.append({
            "xq_t": np.ascontiguousarray(query[b].T).astype(BF16),
            "xk_t": np.ascontiguousarray(key[b].T).astype(BF16),
            "xv_t": np.ascontiguousarray(value[b].T).astype(BF16),
            "w1t": np.ascontiguousarray(W1[gs, :].T).astype(BF16),
            "w2t": np.ascontiguousarray(W2[gs, :].T).astype(BF16),
            "w3i": np.ascontiguousarray(w3i).astype(BF16),
            "w4t": np.ascontiguousarray(W4[:, gs].T).astype(BF16),
            "b1c": np.ascontiguousarray(b1[gs].reshape(F // P, P).T).astype(np.float32),
            "b2c": np.ascontiguousarray(b2[gs].reshape(F // P, P).T).astype(np.float32),
            "b3i": b3i.reshape(1, VW).astype(BF16),
        })
    return in_maps


def kernel(query, key, value, W1, b1, W2, b2, W3, b3, W4, b4, _trace=False, _tmpdir=None):
    args = [np.asarray(a) for a in (query, key, value, W1, b1, W2, b2, W3, b3, W4, b4)]
    nc = _get_nc()
    in_maps = _make_in_maps(*args)
    res = run_bass_kernel_spmd(
        nc, in_maps, core_ids=list(range(N_CORES)),
        trace=_trace, tmpdir=_tmpdir,
    )
    b4_f = args[10].astype(np.float32)
    full = np.zeros((B, S, D), np.float32)
    for c in range(N_CORES):
        full[c // 4] += res.results[c]["out"]
    full += b4_f[None, None, :]
    kernel.last_results = res
    return full


# revision 15
# speedup vs baseline: 1.1964x; 1.1791x over previous
"""Multi-head attention (B=2, S=2048, D=1024, H=16, d_k=64) on 8 NeuronCores.

Sharding: 8 cores = 2 batches x 4 head-groups (4 heads each).
Core c handles batch b = c//4 and heads 4*(c%4) .. 4*(c%4)+4 (feature
slice of width F=256). Each core computes its partial output-projection
contribution [S, D]; the host sums the 4 head-group partials per batch
and adds b4 (the "all-reduce" of the row-sharded W4 projection).

Device dataflow works in a "transposed world" so every matmul operand
is in its natural PE layout (contraction on partitions), with zero
on-device transposes:
  qT = W1g @ x_q.T  [F, S]
  kT = W2g @ x_k.T  [F, S]
  v  = x_v @ W3i    [S, 260]  (260 = 4 heads x (64 v cols + 1 ones col);
                               the ones col comes from the bias matmul
                               with b3i[h*65+64] = 1, W3i zero there)
  scoresT_h = kT_h.T @ qT_h   [S_keys, S_q]  (K = d_k = 64)
  attnT = exp(scoresT / 8)    ACT, PSUM->SBUF bf16, no max subtraction
  pv = v_ext.T @ attnT        [65, S_q]; row 64 = softmax denominator
  outT_h = pv[0:64] * (1/pv[64])   (reciprocal_approx_fast + gpsimd
                                    partition_broadcast + one DVE mul)
  partial = outT.T @ W4g.T    [S, D]

All matmuls bf16 with f32 PSUM accumulation.

Schedule (built to keep the PE stream dense so the HAM clock gate stays
at 8/8, and to hide everything under the ACT exp roofline):
  - prioritized chunked DMA: w1 + xq first (512-col chunks) so the
    first projection matmuls issue ~5us in
  - upfront PE work: only the m=0 half of the q/k projections (enough
    for head-pair 0's windows)
  - window (0,0): scores/exp/PV with the v-projection interleaved as
    PE filler (one s-tile per kt step, sharing the "sc" psum tag)
  - window (0,1): m=1 q/k projection groups as filler
  - window (1,0): no filler
  - window (1,1): W4 output projection for qw=0 as filler
  - tail: W4 for qw=1
  PV lags scores by 2 key tiles so PV never waits on the exp of the
  same step (ACT latency hidden at any clock).  PSUM: scores 2x2 banks
  (tag sc, shared by all filler psum) + PV accumulators 2x2 banks.
"""

import numpy as np
import ml_dtypes

import concourse.bass as bass
import concourse.mybir as mybir
import concourse.tile as tile
from concourse import bacc
from concourse.bass_utils import run_bass_kernel_spmd

BF16 = ml_dtypes.bfloat16
F32 = mybir.dt.float32
BF = mybir.dt.bfloat16

B, S, D = 2, 2048, 1024
H_CORE = 4          # heads per core
DK = 64             # head dim
F = H_CORE * DK     # features per core = 256
P = 128             # partitions
KB = D // P         # k blocks in D contraction = 8
SM = S // P         # seq tiles of 128 = 16
QW = 1024           # query window width
NQW = S // QW       # query windows = 2
VW = H_CORE * (DK + 1)  # 260: v with interleaved ones columns
N_CORES = 8


def _build_kernel():
    nc = bacc.Bacc(
        "TRN2",
        target_bir_lowering=False,
        debug=False,
        enable_asserts=False,
        num_devices=N_CORES,
    )

    xq = nc.dram_tensor("xq_t", [D, S], BF, kind="ExternalInput").ap()
    xk = nc.dram_tensor("xk_t", [D, S], BF, kind="ExternalInput").ap()
    xv = nc.dram_tensor("xv_t", [D, S], BF, kind="ExternalInput").ap()
    w1 = nc.dram_tensor("w1t", [D, F], BF, kind="ExternalInput").ap()
    w2 = nc.dram_tensor("w2t", [D, F], BF, kind="ExternalInput").ap()
    w3 = nc.dram_tensor("w3i", [D, VW], BF, kind="ExternalInput").ap()
    w4 = nc.dram_tensor("w4t", [F, D], BF, kind="ExternalInput").ap()
    b1 = nc.dram_tensor("b1c", [P, F // P], F32, kind="ExternalInput").ap()
    b2 = nc.dram_tensor("b2c", [P, F // P], F32, kind="ExternalInput").ap()
    b3 = nc.dram_tensor("b3i", [1, VW], BF, kind="ExternalInput").ap()
    out = nc.dram_tensor("out", [S, D], BF, kind="ExternalOutput").ap()

    with tile.TileContext(nc) as tc:
        _body(tc, xq, xk, xv, w1, w2, w3, w4, b1, b2, b3, out)

    nc.compile()
    return nc


def _body(tc, xq, xk, xv, w1, w2, w3, w4, b1, b2, b3, out):
    nc = tc.nc
    MF = F // P  # m tiles for the F=256 feature dim = 2

    with (
        tc.tile_pool(name="wpool", bufs=1) as wpool,
        tc.tile_pool(name="xt", bufs=24) as xt_pool,
        tc.tile_pool(name="persist", bufs=1) as persist,
        tc.tile_pool(name="attn", bufs=6) as attn_pool,
        tc.tile_pool(name="small", bufs=3) as small,
        tc.tile_pool(name="stage", bufs=2) as stage,
        tc.tile_pool(name="psum", bufs=1, space="PSUM") as psum,
    ):
        # ---- SBUF tiles; DMA issued in priority order ----
        w1_sb = [wpool.tile([P, F], BF, name=f"w1_{k}", tag=f"w1_{k}") for k in range(KB)]
        w2_sb = [wpool.tile([P, F], BF, name=f"w2_{k}", tag=f"w2_{k}") for k in range(KB)]
        w3_sb = [wpool.tile([P, VW], BF, name=f"w3_{k}", tag=f"w3_{k}") for k in range(KB)]
        w4_sb = [wpool.tile([P, D], BF, name=f"w4_{k}", tag=f"w4_{k}") for k in range(MF)]
        xq_sb = [xt_pool.tile([P, S], BF, name=f"xq_{k}", tag="xt") for k in range(KB)]
        xk_sb = [xt_pool.tile([P, S], BF, name=f"xk_{k}", tag="xt") for k in range(KB)]
        xv_sb = [xt_pool.tile([P, S], BF, name=f"xv_{k}", tag="xt") for k in range(KB)]
        b1_sb = wpool.tile([P, MF], F32, name="b1_sb", tag="b1_sb")
        b2_sb = wpool.tile([P, MF], F32, name="b2_sb", tag="b2_sb")
        b3_sb = wpool.tile([1, VW], BF, name="b3_sb", tag="b3_sb")
        ones_row = wpool.tile([1, P], BF, name="ones_row", tag="ones_row")
        nc.vector.memset(ones_row[:], 1.0)

        # two parallel HWDGE queues: sync carries xq+xk (the critical path),
        # scalar carries weights + xv + biases (scalar is idle pre-attention)
        for k in range(KB):
            nc.scalar.dma_start(w1_sb[k][:], w1[k * P:(k + 1) * P, :])
        for k in range(KB):
            nc.sync.dma_start(xq_sb[k][:], xq[k * P:(k + 1) * P, :])
        for k in range(KB):
            nc.scalar.dma_start(xv_sb[k][:], xv[k * P:(k + 1) * P, :])
        for k in range(KB):
            nc.scalar.dma_start(w2_sb[k][:], w2[k * P:(k + 1) * P, :])
        for k in range(KB):
            nc.scalar.dma_start(w3_sb[k][:], w3[k * P:(k + 1) * P, :])
        for k in range(KB):
            nc.sync.dma_start(xk_sb[k][:], xk[k * P:(k + 1) * P, :])
        for k in range(MF):
            nc.scalar.dma_start(w4_sb[k][:], w4[k * P:(k + 1) * P, :])
        nc.scalar.dma_start(b1_sb[:], b1[:])
        nc.scalar.dma_start(b2_sb[:], b2[:])
        nc.scalar.dma_start(b3_sb[:], b3[:])

        # persistent activations
        qT = [persist.tile([P, S], BF, name=f"qT_{m}", tag=f"qT_{m}") for m in range(MF)]
        kT = [persist.tile([P, S], BF, name=f"kT_{m}", tag=f"kT_{m}") for m in range(MF)]
        v_sb = [persist.tile([P, VW], BF, name=f"v_{s}", tag=f"v_{s}") for s in range(SM)]
        outT = [persist.tile([P, S], BF, name=f"outT_{m}", tag=f"outT_{m}") for m in range(MF)]

        # ---- one q/k projection group: dst[m][:, 512-col slice] ----
        def proj_qk(name, x_sb, w_sb, b_sb, dst, m, h):
            csl = slice(h * 512, (h + 1) * 512)
            ps = psum.tile([P, 512], F32, name=f"pp_{name}_{m}_{h}", tag="sc", bufs=2)
            for k in range(KB):
                nc.tensor.matmul(
                    ps[:],
                    w_sb[k][:, m * P:(m + 1) * P],
                    x_sb[k][:, csl],
                    start=(k == 0),
                    stop=(k == KB - 1),
                )
            nc.vector.tensor_scalar_add(dst[m][:, csl], ps[:], b_sb[:, m:m + 1])

        # upfront: m=0 projections of q and k, k-major over 4 concurrent
        # [P,512] psum groups so each x tile is consumed as its DMA lands
        def proj_m0_kmajor(name, x_sb, w_sb, b_sb, dst):
            tags = ["sc", "sc", "pv", "pv"]
            pss = [
                psum.tile([P, 512], F32, name=f"pp0_{name}_{h}", tag=tags[h], bufs=2)
                for h in range(4)
            ]
            for k in range(KB):
                for h in range(4):
                    nc.tensor.matmul(
                        pss[h][:],
                        w_sb[k][:, 0:P],
                        x_sb[k][:, h * 512:(h + 1) * 512],
                        start=(k == 0),
                        stop=(k == KB - 1),
                    )
            for h in range(4):
                nc.vector.tensor_scalar_add(
                    dst[0][:, h * 512:(h + 1) * 512], pss[h][:], b_sb[:, 0:1]
                )

        proj_m0_kmajor("q", xq_sb, w1_sb, b1_sb, qT)
        # m=1 q projection here: dense PE work that only needs xq, covering
        # the wait for the xk DMA before the k projections can run
        for h in range(4):
            proj_qk("q1", xq_sb, w1_sb, b1_sb, qT, 1, h)
        proj_m0_kmajor("k", xk_sb, w2_sb, b2_sb, kT)

        # ---- filler generators (consumed inside windows at PE idle slots) ----
        def gen_vproj():
            for s in range(SM):
                ps = psum.tile([P, VW], F32, name=f"pv_{s}", tag="sc", bufs=2)
                for k in range(KB):
                    nc.tensor.matmul(
                        ps[:],
                        xv_sb[k][:, s * P:(s + 1) * P],
                        w3_sb[k][:],
                        start=(k == 0),
                        stop=False,
                    )
                nc.tensor.matmul(ps[:], ones_row[:], b3_sb[:], start=False, stop=True)
                nc.vector.tensor_copy(v_sb[s][:], ps[:])
                yield

        def gen_m1proj():
            for h in range(4):
                proj_qk("k1", xk_sb, w2_sb, b2_sb, kT, 1, h)
                yield

        def gen_w4(qts, alt_copy=False):
            for i, qt in enumerate(qts):
                ps = psum.tile([P, D], F32, name=f"po_{qt}", tag="sc", bufs=2)
                for oc in range(D // 512):
                    for m in range(MF):
                        nc.tensor.matmul(
                            ps[:, oc * 512:(oc + 1) * 512],
                            outT[m][:, qt * P:(qt + 1) * P],
                            w4_sb[m][:, oc * 512:(oc + 1) * 512],
                            start=(m == 0),
                            stop=(m == MF - 1),
                        )
                ob = stage.tile([P, D], BF, name=f"ob_{qt}", tag="ob")
                if alt_copy and i % 2 == 1:
                    nc.scalar.copy(ob[:], ps[:])
                else:
                    nc.vector.tensor_copy(ob[:], ps[:])
                nc.sync.dma_start(out[qt * P:(qt + 1) * P, :], ob[:])
                yield

        # ---- attention window: head-pair hp, query window qw.
        #      scores(kt) / PV(kt-2) interleave; filler consumed each step. ----
        def window(hp, qw, filler=None, fill_every=1, fill_start=0, drain=False):
            qsl = slice(qw * QW, (qw + 1) * QW)
            attn_t = [[None] * SM for _ in range(2)]
            pv_ps = [
                psum.tile([P, QW], F32, name=f"pvps_{hp}_{qw}_{h2}", tag="pv", bufs=2)
                for h2 in range(2)
            ]

            def emit_scores(kt):
                for h2 in range(2):
                    rsl = slice(h2 * DK, (h2 + 1) * DK)
                    ps = psum.tile([P, QW], F32, name=f"sc_{hp}_{qw}_{kt}_{h2}",
                                   tag="sc", bufs=2)
                    for half in range(2):
                        nc.tensor.matmul(
                            ps[:, half * 512:(half + 1) * 512],
                            kT[hp][rsl, kt * P:(kt + 1) * P],
                            qT[hp][rsl, qw * QW + half * 512: qw * QW + (half + 1) * 512],
                            start=True,
                            stop=True,
                        )
                    at = attn_pool.tile([P, QW], BF, name=f"at_{hp}_{qw}_{kt}_{h2}",
                                        tag="attnT", bufs=6)
                    nc.scalar.activation(
                        at[:], ps[:], mybir.ActivationFunctionType.Exp,
                        scale=1.0 / np.sqrt(DK),
                    )
                    attn_t[h2][kt] = at

            def emit_pv(kt):
                for h2 in range(2):
                    h = hp * 2 + h2
                    vsl = slice(h * (DK + 1), h * (DK + 1) + DK + 1)
                    for half in range(2):
                        nc.tensor.matmul(
                            pv_ps[h2][0:DK + 1, half * 512:(half + 1) * 512],
                            v_sb[kt][:, vsl],
                            attn_t[h2][kt][:, half * 512:(half + 1) * 512],
                            start=(kt == 0),
                            stop=(kt == SM - 1),
                        )

            for kt in range(SM):
                emit_scores(kt)
                if kt >= 2:
                    emit_pv(kt - 2)
                if (filler is not None and kt >= fill_start
                        and (kt - fill_start) % fill_every == 0):
                    next(filler, None)
            emit_pv(SM - 2)
            emit_pv(SM - 1)

            # normalization in column halves (shorter dependency chains at
            # window boundaries / tail): den copy -> fast reciprocal ->
            # gpsimd partition broadcast -> one DVE multiply into outT.
            for half in range(2):
                hsl = slice(half * 512, (half + 1) * 512)
                osl = slice(qw * QW + half * 512, qw * QW + (half + 1) * 512)
                for h2 in range(2):
                    den = small.tile([1, 512], F32, name=f"den_{hp}_{qw}_{h2}_{half}",
                                     tag="den", bufs=3)
                    nc.vector.tensor_copy(den[:], pv_ps[h2][DK:DK + 1, hsl])
                    rec = small.tile([1, 512], F32, name=f"rec_{hp}_{qw}_{h2}_{half}",
                                     tag="rec", bufs=3)
                    nc.vector.reciprocal_approx_fast(rec[:], den[:])
                    bc = small.tile([DK, 512], F32, name=f"bc_{hp}_{qw}_{h2}_{half}",
                                    tag="bc", bufs=2)
                    nc.gpsimd.partition_broadcast(bc[:], rec[:])
                    nc.vector.tensor_mul(
                        outT[hp][h2 * DK:(h2 + 1) * DK, osl], pv_ps[h2][0:DK, hsl], bc[:]
                    )
            if drain and filler is not None:
                for _ in filler:
                    pass

        window(0, 0, filler=gen_vproj(), fill_every=1)
        window(0, 1, filler=gen_m1proj(), fill_every=2, fill_start=1)
        window(1, 0)
        window(1, 1, filler=gen_w4(range(SM // 2)), fill_every=2, fill_start=2,
               drain=True)
        for _ in gen_w4(range(SM // 2, SM), alt_copy=True):
            pass


_NC_CACHE = None


def _get_nc():
    global _NC_CACHE
    if _NC_CACHE is None:
        _NC_CACHE = _build_kernel()
    return _NC_CACHE


def _make_in_maps(query, key, value, W1, b1, W2, b2, W3, b3, W4, b4):
    in_maps = []
    for c in range(N_CORES):
        b, g = divmod(c, 4)
        gs = slice(g * F, (g + 1) * F)
        w3g = W3[gs, :].T.astype(np.float32)          # [D, F]
        w3i = np.zeros((D, VW), np.float32)
        b3g = b3[gs].astype(np.float32)
        b3i = np.zeros((VW,), np.float32)
        for h in range(H_CORE):
            w3i[:, h * (DK + 1): h * (DK + 1) + DK] = w3g[:, h * DK:(h + 1) * DK]
            b3i[h * (DK + 1): h * (DK + 1) + DK] = b3g[h * DK:(h + 1) * DK]
            b3i[h * (DK + 1) + DK] = 1.0
        in_maps.append({
            "xq_t": np.ascontiguousarray(query[b].T).astype(BF16),
            "xk_t": np.ascontiguousarray(key[b].T).astype(BF16),
            "xv_t": np.ascontiguousarray(value[b].T).astype(BF16),
            "w1t": np.ascontiguousarray(W1[gs, :].T).astype(BF16),
            "w2t": np.ascontiguousarray(W2[gs, :].T).astype(BF16),
            "w3i": np.ascontiguousarray(w3i).astype(BF16),
            "w4t": np.ascontiguousarray(W4[:, gs].T).astype(BF16),
            "b1c": np.ascontiguousarray(b1[gs].reshape(F // P, P).T).astype(np.float32),
            "b2c": np.ascontiguousarray(b2[gs].reshape(F // P, P).T).astype(np.float32),
            "b3i": b3i.reshape(1, VW).astype(BF16),
        })
    return in_maps


def kernel(query, key, value, W1, b1, W2, b2, W3, b3, W4, b4, _trace=False, _tmpdir=None):
    args = [np.asarray(a) for a in (query, key, value, W1, b1, W2, b2, W3, b3, W4, b4)]
    nc = _get_nc()
    in_maps = _make_in_maps(*args)
    res = run_bass_kernel_spmd(
        nc, in_maps, core_ids=list(range(N_CORES)),
        trace=_trace, tmpdir=_tmpdir,
    )
    b4_f = args[10].astype(np.float32)
    full = np.zeros((B, S, D), np.float32)
    for c in range(N_CORES):
        full[c // 4] += res.results[c]["out"]
    full += b4_f[None, None, :]
    kernel.last_results = res
    return full


# revision 20
# speedup vs baseline: 1.2283x; 1.0266x over previous
"""Multi-head attention (B=2, S=2048, D=1024, H=16, d_k=64) on 8 NeuronCores.

Sharding: 8 cores = 2 batches x 4 head-groups (4 heads each).
Core c handles batch b = c//4 and heads 4*(c%4) .. 4*(c%4)+4 (feature
slice of width F=256). Each core computes its partial output-projection
contribution [S, D]; the host sums the 4 head-group partials per batch
and adds b4 (the "all-reduce" of the row-sharded W4 projection).

Device dataflow works in a "transposed world" so every matmul operand
is in its natural PE layout (contraction on partitions), with zero
on-device transposes:
  qT = W1g @ x_q.T  [F, S]
  kT = W2g @ x_k.T  [F, S]
  v  = x_v @ W3i    [S, 260]  (260 = 4 heads x (64 v cols + 1 ones col);
                               the ones col comes from the bias matmul
                               with b3i[h*65+64] = 1, W3i zero there)
  scoresT_h = kT_h.T @ qT_h   [S_keys, S_q]  (K = d_k = 64)
  attnT = exp(scoresT / 8)    ACT, PSUM->SBUF bf16, no max subtraction
  pv = v_ext.T @ attnT        [65, S_q]; row 64 = softmax denominator
  outT_h = pv[0:64] * (1/pv[64])   (reciprocal_approx_fast + gpsimd
                                    partition_broadcast + one DVE mul)
  partial = outT.T @ W4g.T    [S, D]

All matmuls bf16 with f32 PSUM accumulation.

Schedule (built to keep the PE stream dense so the HAM clock gate stays
at 8/8, and to hide everything under the ACT exp roofline):
  - prioritized chunked DMA: w1 + xq first (512-col chunks) so the
    first projection matmuls issue ~5us in
  - upfront PE work: only the m=0 half of the q/k projections (enough
    for head-pair 0's windows)
  - window (0,0): scores/exp/PV with the v-projection interleaved as
    PE filler (one s-tile per kt step, sharing the "sc" psum tag)
  - window (0,1): m=1 q/k projection groups as filler
  - window (1,0): no filler
  - window (1,1): W4 output projection for qw=0 as filler
  - tail: W4 for qw=1
  PV lags scores by 2 key tiles so PV never waits on the exp of the
  same step (ACT latency hidden at any clock).  PSUM: scores 2x2 banks
  (tag sc, shared by all filler psum) + PV accumulators 2x2 banks.
"""

import numpy as np
import ml_dtypes

import concourse.bass as bass
import concourse.mybir as mybir
import concourse.tile as tile
from concourse import bacc
from concourse.bass_utils import run_bass_kernel_spmd

BF16 = ml_dtypes.bfloat16
F32 = mybir.dt.float32
BF = mybir.dt.bfloat16

B, S, D = 2, 2048, 1024
H_CORE = 4          # heads per core
DK = 64             # head dim
F = H_CORE * DK     # features per core = 256
P = 128             # partitions
KB = D // P         # k blocks in D contraction = 8
SM = S // P         # seq tiles of 128 = 16
QW = 1024           # query window width
NQW = S // QW       # query windows = 2
VW = H_CORE * (DK + 1)  # 260: v with interleaved ones columns
N_CORES = 8


def _build_kernel():
    nc = bacc.Bacc(
        "TRN2",
        target_bir_lowering=False,
        debug=False,
        enable_asserts=False,
        num_devices=N_CORES,
    )

    xq = nc.dram_tensor("xq_t", [D, S], BF, kind="ExternalInput").ap()
    xk = nc.dram_tensor("xk_t", [D, S], BF, kind="ExternalInput").ap()
    xv = nc.dram_tensor("xv_t", [D, S], BF, kind="ExternalInput").ap()
    w1 = nc.dram_tensor("w1t", [D, F], BF, kind="ExternalInput").ap()
    w2 = nc.dram_tensor("w2t", [D, F], BF, kind="ExternalInput").ap()
    w3 = nc.dram_tensor("w3i", [D, VW], BF, kind="ExternalInput").ap()
    w4 = nc.dram_tensor("w4t", [F, D], BF, kind="ExternalInput").ap()
    b1 = nc.dram_tensor("b1c", [P, F // P], F32, kind="ExternalInput").ap()
    b2 = nc.dram_tensor("b2c", [P, F // P], F32, kind="ExternalInput").ap()
    b3 = nc.dram_tensor("b3i", [1, VW], BF, kind="ExternalInput").ap()
    out = nc.dram_tensor("out", [S, D], BF, kind="ExternalOutput").ap()

    with tile.TileContext(nc) as tc:
        _body(tc, xq, xk, xv, w1, w2, w3, w4, b1, b2, b3, out)

    nc.compile()
    return nc


def _body(tc, xq, xk, xv, w1, w2, w3, w4, b1, b2, b3, out):
    nc = tc.nc
    MF = F // P  # m tiles for the F=256 feature dim = 2

    with (
        tc.tile_pool(name="wpool", bufs=1) as wpool,
        tc.tile_pool(name="xt", bufs=24) as xt_pool,
        tc.tile_pool(name="persist", bufs=1) as persist,
        tc.tile_pool(name="attn", bufs=10) as attn_pool,
        tc.tile_pool(name="small", bufs=3) as small,
        tc.tile_pool(name="stage", bufs=2) as stage,
        tc.tile_pool(name="psum", bufs=1, space="PSUM") as psum,
    ):
        # ---- SBUF tiles; DMA issued in priority order ----
        w1_sb = [wpool.tile([P, F], BF, name=f"w1_{k}", tag=f"w1_{k}") for k in range(KB)]
        w2_sb = [wpool.tile([P, F], BF, name=f"w2_{k}", tag=f"w2_{k}") for k in range(KB)]
        w3_sb = [wpool.tile([P, VW], BF, name=f"w3_{k}", tag=f"w3_{k}") for k in range(KB)]
        w4_sb = [wpool.tile([P, D], BF, name=f"w4_{k}", tag=f"w4_{k}") for k in range(MF)]
        xq_sb = [xt_pool.tile([P, S], BF, name=f"xq_{k}", tag="xt") for k in range(KB)]
        xk_sb = [xt_pool.tile([P, S], BF, name=f"xk_{k}", tag="xt") for k in range(KB)]
        xv_sb = [xt_pool.tile([P, S], BF, name=f"xv_{k}", tag="xt") for k in range(KB)]
        b1_sb = wpool.tile([P, MF], F32, name="b1_sb", tag="b1_sb")
        b2_sb = wpool.tile([P, MF], F32, name="b2_sb", tag="b2_sb")
        b3_sb = wpool.tile([1, VW], BF, name="b3_sb", tag="b3_sb")
        ones_row = wpool.tile([1, P], BF, name="ones_row", tag="ones_row")
        nc.vector.memset(ones_row[:], 1.0)

        # two parallel HWDGE queues: sync carries the big x tensors in
        # criticality order, scalar carries all weights + biases
        for k in range(KB):
            nc.scalar.dma_start(w1_sb[k][:], w1[k * P:(k + 1) * P, :])
        for k in range(KB):
            nc.sync.dma_start(xq_sb[k][:], xq[k * P:(k + 1) * P, :])
        for k in range(KB):
            nc.scalar.dma_start(w2_sb[k][:], w2[k * P:(k + 1) * P, :])
        for k in range(KB):
            nc.scalar.dma_start(w3_sb[k][:], w3[k * P:(k + 1) * P, :])
        for k in range(KB):
            nc.sync.dma_start(xk_sb[k][:], xk[k * P:(k + 1) * P, :])
        for k in range(KB):
            nc.sync.dma_start(xv_sb[k][:], xv[k * P:(k + 1) * P, :])
        for k in range(MF):
            nc.scalar.dma_start(w4_sb[k][:], w4[k * P:(k + 1) * P, :])
        nc.scalar.dma_start(b1_sb[:], b1[:])
        nc.scalar.dma_start(b2_sb[:], b2[:])
        nc.scalar.dma_start(b3_sb[:], b3[:])

        # persistent activations
        qT = [persist.tile([P, S], BF, name=f"qT_{m}", tag=f"qT_{m}") for m in range(MF)]
        kT = [persist.tile([P, S], BF, name=f"kT_{m}", tag=f"kT_{m}") for m in range(MF)]
        v_sb = [persist.tile([P, VW], BF, name=f"v_{s}", tag=f"v_{s}") for s in range(SM)]
        outT = [persist.tile([P, S], BF, name=f"outT_{m}", tag=f"outT_{m}") for m in range(MF)]

        # ---- one q/k projection group: dst[m][:, 512-col slice] ----
        def proj_qk(name, x_sb, w_sb, b_sb, dst, m, h):
            csl = slice(h * 512, (h + 1) * 512)
            ps = psum.tile([P, 512], F32, name=f"pp_{name}_{m}_{h}", tag="sc", bufs=2)
            for k in range(KB):
                nc.tensor.matmul(
                    ps[:],
                    w_sb[k][:, m * P:(m + 1) * P],
                    x_sb[k][:, csl],
                    start=(k == 0),
                    stop=(k == KB - 1),
                )
            nc.vector.tensor_scalar_add(dst[m][:, csl], ps[:], b_sb[:, m:m + 1])

        # upfront: m=0 projections of q and k, k-major over 4 concurrent
        # [P,512] psum groups so each x tile is consumed as its DMA lands
        def proj_m0_kmajor(name, x_sb, w_sb, b_sb, dst):
            tags = ["sc", "sc", "pv", "pv"]
            pss = [
                psum.tile([P, 512], F32, name=f"pp0_{name}_{h}", tag=tags[h], bufs=2)
                for h in range(4)
            ]
            for k in range(KB):
                for h in range(4):
                    nc.tensor.matmul(
                        pss[h][:],
                        w_sb[k][:, 0:P],
                        x_sb[k][:, h * 512:(h + 1) * 512],
                        start=(k == 0),
                        stop=(k == KB - 1),
                    )
            for h in range(4):
                nc.vector.tensor_scalar_add(
                    dst[0][:, h * 512:(h + 1) * 512], pss[h][:], b_sb[:, 0:1]
                )

        proj_m0_kmajor("q", xq_sb, w1_sb, b1_sb, qT)
        # m=1 q projection here: dense PE work that only needs xq, covering
        # the wait for the xk DMA before the k projections can run
        for h in range(4):
            proj_qk("q1", xq_sb, w1_sb, b1_sb, qT, 1, h)
        proj_m0_kmajor("k", xk_sb, w2_sb, b2_sb, kT)

        # ---- filler generators (consumed inside windows at PE idle slots) ----
        def gen_vproj():
            for s in range(SM):
                ps = psum.tile([P, VW], F32, name=f"pv_{s}", tag="sc", bufs=2)
                for k in range(KB):
                    nc.tensor.matmul(
                        ps[:],
                        xv_sb[k][:, s * P:(s + 1) * P],
                        w3_sb[k][:],
                        start=(k == 0),
                        stop=False,
                    )
                nc.tensor.matmul(ps[:], ones_row[:], b3_sb[:], start=False, stop=True)
                nc.vector.tensor_copy(v_sb[s][:], ps[:])
                yield

        def gen_m1proj():
            for h in range(4):
                proj_qk("k1", xk_sb, w2_sb, b2_sb, kT, 1, h)
                yield

        def gen_w4(qts, alt_copy=False):
            for i, qt in enumerate(qts):
                ps = psum.tile([P, D], F32, name=f"po_{qt}", tag="sc", bufs=2)
                for oc in range(D // 512):
                    for m in range(MF):
                        nc.tensor.matmul(
                            ps[:, oc * 512:(oc + 1) * 512],
                            outT[m][:, qt * P:(qt + 1) * P],
                            w4_sb[m][:, oc * 512:(oc + 1) * 512],
                            start=(m == 0),
                            stop=(m == MF - 1),
                        )
                ob = stage.tile([P, D], BF, name=f"ob_{qt}", tag="ob")
                if alt_copy and i % 2 == 1:
                    nc.scalar.copy(ob[:], ps[:])
                else:
                    nc.vector.tensor_copy(ob[:], ps[:])
                nc.sync.dma_start(out[qt * P:(qt + 1) * P, :], ob[:])
                yield

        # ---- attention window: head-pair hp, query window qw.
        #      scores(kt) / PV(kt-2) interleave; filler consumed each step. ----
        LAG = 4  # PV trails scores by LAG key tiles (hides exp + v-fill latency)

        def window(hp, qw, filler=None, fill_every=1, fill_start=0, drain=False):
            qsl = slice(qw * QW, (qw + 1) * QW)
            attn_t = [[None] * SM for _ in range(2)]
            pv_ps = [
                psum.tile([P, QW], F32, name=f"pvps_{hp}_{qw}_{h2}", tag="pv", bufs=2)
                for h2 in range(2)
            ]

            def emit_scores(kt):
                for h2 in range(2):
                    rsl = slice(h2 * DK, (h2 + 1) * DK)
                    ps = psum.tile([P, QW], F32, name=f"sc_{hp}_{qw}_{kt}_{h2}",
                                   tag="sc", bufs=2)
                    for half in range(2):
                        nc.tensor.matmul(
                            ps[:, half * 512:(half + 1) * 512],
                            kT[hp][rsl, kt * P:(kt + 1) * P],
                            qT[hp][rsl, qw * QW + half * 512: qw * QW + (half + 1) * 512],
                            start=True,
                            stop=True,
                        )
                    at = attn_pool.tile([P, QW], BF, name=f"at_{hp}_{qw}_{kt}_{h2}",
                                        tag="attnT", bufs=10)
                    nc.scalar.activation(
                        at[:], ps[:], mybir.ActivationFunctionType.Exp,
                        scale=1.0 / np.sqrt(DK),
                    )
                    attn_t[h2][kt] = at

            def emit_pv(kt):
                for h2 in range(2):
                    h = hp * 2 + h2
                    vsl = slice(h * (DK + 1), h * (DK + 1) + DK + 1)
                    for half in range(2):
                        nc.tensor.matmul(
                            pv_ps[h2][0:DK + 1, half * 512:(half + 1) * 512],
                            v_sb[kt][:, vsl],
                            attn_t[h2][kt][:, half * 512:(half + 1) * 512],
                            start=(kt == 0),
                            stop=(kt == SM - 1),
                        )

            for kt in range(SM):
                emit_scores(kt)
                if (filler is not None and kt >= fill_start
                        and (kt - fill_start) % fill_every == 0):
                    next(filler, None)
                if kt >= LAG:
                    emit_pv(kt - LAG)
            if drain and filler is not None:
                for _ in filler:
                    pass
            for kt in range(SM - LAG, SM):
                emit_pv(kt)

            # normalization in column halves (shorter dependency chains at
            # window boundaries / tail): den copy -> fast reciprocal ->
            # gpsimd partition broadcast -> one DVE multiply into outT.
            for half in range(2):
                hsl = slice(half * 512, (half + 1) * 512)
                osl = slice(qw * QW + half * 512, qw * QW + (half + 1) * 512)
                for h2 in range(2):
                    den = small.tile([1, 512], F32, name=f"den_{hp}_{qw}_{h2}_{half}",
                                     tag="den", bufs=3)
                    nc.vector.tensor_copy(den[:], pv_ps[h2][DK:DK + 1, hsl])
                    rec = small.tile([1, 512], F32, name=f"rec_{hp}_{qw}_{h2}_{half}",
                                     tag="rec", bufs=3)
                    nc.vector.reciprocal_approx_fast(rec[:], den[:])
                    bc = small.tile([DK, 512], F32, name=f"bc_{hp}_{qw}_{h2}_{half}",
                                    tag="bc", bufs=2)
                    nc.gpsimd.partition_broadcast(bc[:], rec[:])
                    nc.vector.tensor_mul(
                        outT[hp][h2 * DK:(h2 + 1) * DK, osl], pv_ps[h2][0:DK, hsl], bc[:]
                    )

        window(0, 0, filler=gen_vproj(), fill_every=1, fill_start=2, drain=True)
        window(0, 1, filler=gen_m1proj(), fill_every=2, fill_start=1)
        window(1, 0)
        window(1, 1, filler=gen_w4(range(SM // 2)), fill_every=2, fill_start=2,
               drain=True)
        for _ in gen_w4(range(SM // 2, SM), alt_copy=True):
            pass


_NC_CACHE = None


def _get_nc():
    global _NC_CACHE
    if _NC_CACHE is None:
        _NC_CACHE = _build_kernel()
    return _NC_CACHE


def _make_in_maps(query, key, value, W1, b1, W2, b2, W3, b3, W4, b4):
    in_maps = []
    for c in range(N_CORES):
        b, g = divmod(c, 4)
        gs = slice(g * F, (g + 1) * F)
        w3g = W3[gs, :].T.astype(np.float32)          # [D, F]
        w3i = np.zeros((D, VW), np.float32)
        b3g = b3[gs].astype(np.float32)
        b3i = np.zeros((VW,), np.float32)
        for h in range(H_CORE):
            w3i[:, h * (DK + 1): h * (DK + 1) + DK] = w3g[:, h * DK:(h + 1) * DK]
            b3i[h * (DK + 1): h * (DK + 1) + DK] = b3g[h * DK:(h + 1) * DK]
            b3i[h * (DK + 1) + DK] = 1.0
        in_maps.append({
            "xq_t": np.ascontiguousarray(query[b].T).astype(BF16),
            "xk_t": np.ascontiguousarray(key[b].T).astype(BF16),
            "xv_t": np.ascontiguousarray(value[b].T).astype(BF16),
            "w1t": np.ascontiguousarray(W1[gs, :].T).astype(BF16),
            "w2t": np.ascontiguousarray(W2[gs, :].T).astype(BF16),
            "w3i": np.ascontiguousarray(w3i).astype(BF16),
            "w4t": np.ascontiguousarray(W4[:, gs].T).astype(BF16),
            "b1c": np.ascontiguousarray(b1[gs].reshape(F // P, P).T).astype(np.float32),
            "b2c": np.ascontiguousarray(b2[gs].reshape(F // P, P).T).astype(np.float32),
            "b3i": b3i.reshape(1, VW).astype(BF16),
        })
    return in_maps


def kernel(query, key, value, W1, b1, W2, b2, W3, b3, W4, b4, _trace=False, _tmpdir=None):
    args = [np.asarray(a) for a in (query, key, value, W1, b1, W2, b2, W3, b3, W4, b4)]
    nc = _get_nc()
    in_maps = _make_in_maps(*args)
    res = run_bass_kernel_spmd(
        nc, in_maps, core_ids=list(range(N_CORES)),
        trace=_trace, tmpdir=_tmpdir,
    )
    b4_f = args[10].astype(np.float32)
    full = np.zeros((B, S, D), np.float32)
    for c in range(N_CORES):
        full[c // 4] += res.results[c]["out"]
    full += b4_f[None, None, :]
    kernel.last_results = res
    return full


# revision 22
# speedup vs baseline: 1.2489x; 1.0168x over previous
"""Multi-head attention (B=2, S=2048, D=1024, H=16, d_k=64) on 8 NeuronCores.

Sharding: 8 cores = 2 batches x 4 head-groups (4 heads each).
Core c handles batch b = c//4 and heads 4*(c%4) .. 4*(c%4)+4 (feature
slice of width F=256). Each core computes its partial output-projection
contribution [S, D]; the host sums the 4 head-group partials per batch
and adds b4 (the "all-reduce" of the row-sharded W4 projection).

Device dataflow works in a "transposed world" so every matmul operand
is in its natural PE layout (contraction on partitions), with zero
on-device transposes:
  qT = W1g @ x_q.T  [F, S]
  kT = W2g @ x_k.T  [F, S]
  v  = x_v @ W3i    [S, 260]  (260 = 4 heads x (64 v cols + 1 ones col);
                               the ones col comes from the bias matmul
                               with b3i[h*65+64] = 1, W3i zero there)
  scoresT_h = kT_h.T @ qT_h   [S_keys, S_q]  (K = d_k = 64)
  attnT = exp(scoresT / 8)    ACT, PSUM->SBUF bf16, no max subtraction
  pv = v_ext.T @ attnT        [65, S_q]; row 64 = softmax denominator
  outT_h = pv[0:64] * (1/pv[64])   (reciprocal_approx_fast + gpsimd
                                    partition_broadcast + one DVE mul)
  partial = outT.T @ W4g.T    [S, D]

All matmuls bf16 with f32 PSUM accumulation.

Schedule (built to keep the PE stream dense so the HAM clock gate stays
at 8/8, and to hide everything under the ACT exp roofline):
  - prioritized chunked DMA: w1 + xq first (512-col chunks) so the
    first projection matmuls issue ~5us in
  - upfront PE work: only the m=0 half of the q/k projections (enough
    for head-pair 0's windows)
  - window (0,0): scores/exp/PV with the v-projection interleaved as
    PE filler (one s-tile per kt step, sharing the "sc" psum tag)
  - window (0,1): m=1 q/k projection groups as filler
  - window (1,0): no filler
  - window (1,1): W4 output projection for qw=0 as filler
  - tail: W4 for qw=1
  PV lags scores by 2 key tiles so PV never waits on the exp of the
  same step (ACT latency hidden at any clock).  PSUM: scores 2x2 banks
  (tag sc, shared by all filler psum) + PV accumulators 2x2 banks.
"""

import numpy as np
import ml_dtypes

import concourse.bass as bass
import concourse.mybir as mybir
import concourse.tile as tile
from concourse import bacc
from concourse.bass_utils import run_bass_kernel_spmd

BF16 = ml_dtypes.bfloat16
F32 = mybir.dt.float32
BF = mybir.dt.bfloat16

B, S, D = 2, 2048, 1024
H_CORE = 4          # heads per core
DK = 64             # head dim
F = H_CORE * DK     # features per core = 256
P = 128             # partitions
KB = D // P         # k blocks in D contraction = 8
SM = S // P         # seq tiles of 128 = 16
QW = 1024           # query window width
NQW = S // QW       # query windows = 2
VW = H_CORE * (DK + 1)  # 260: v with interleaved ones columns
N_CORES = 8


def _build_kernel():
    nc = bacc.Bacc(
        "TRN2",
        target_bir_lowering=False,
        debug=False,
        enable_asserts=False,
        num_devices=N_CORES,
    )

    xq = nc.dram_tensor("xq_t", [D, S], BF, kind="ExternalInput").ap()
    xk = nc.dram_tensor("xk_t", [D, S], BF, kind="ExternalInput").ap()
    xv = nc.dram_tensor("xv_t", [D, S], BF, kind="ExternalInput").ap()
    w1 = nc.dram_tensor("w1t", [D, F], BF, kind="ExternalInput").ap()
    w2 = nc.dram_tensor("w2t", [D, F], BF, kind="ExternalInput").ap()
    w3 = nc.dram_tensor("w3i", [D, VW], BF, kind="ExternalInput").ap()
    w4 = nc.dram_tensor("w4t", [F, D], BF, kind="ExternalInput").ap()
    b1 = nc.dram_tensor("b1c", [P, F // P], F32, kind="ExternalInput").ap()
    b2 = nc.dram_tensor("b2c", [P, F // P], F32, kind="ExternalInput").ap()
    b3 = nc.dram_tensor("b3i", [1, VW], BF, kind="ExternalInput").ap()
    out = nc.dram_tensor("out", [S, D], BF, kind="ExternalOutput").ap()

    with tile.TileContext(nc) as tc:
        _body(tc, xq, xk, xv, w1, w2, w3, w4, b1, b2, b3, out)

    nc.compile()
    return nc


def _body(tc, xq, xk, xv, w1, w2, w3, w4, b1, b2, b3, out):
    nc = tc.nc
    MF = F // P  # m tiles for the F=256 feature dim = 2

    with (
        tc.tile_pool(name="wpool", bufs=1) as wpool,
        tc.tile_pool(name="xt", bufs=24) as xt_pool,
        tc.tile_pool(name="persist", bufs=1) as persist,
        tc.tile_pool(name="attn", bufs=10) as attn_pool,
        tc.tile_pool(name="small", bufs=3) as small,
        tc.tile_pool(name="stage", bufs=2) as stage,
        tc.tile_pool(name="psum", bufs=1, space="PSUM") as psum,
    ):
        # ---- SBUF tiles; DMA issued in priority order ----
        w1_sb = [wpool.tile([P, F], BF, name=f"w1_{k}", tag=f"w1_{k}") for k in range(KB)]
        w2_sb = [wpool.tile([P, F], BF, name=f"w2_{k}", tag=f"w2_{k}") for k in range(KB)]
        w3_sb = [wpool.tile([P, VW], BF, name=f"w3_{k}", tag=f"w3_{k}") for k in range(KB)]
        w4_sb = [wpool.tile([P, D], BF, name=f"w4_{k}", tag=f"w4_{k}") for k in range(MF)]
        xq_sb = [xt_pool.tile([P, S], BF, name=f"xq_{k}", tag="xt") for k in range(KB)]
        xk_sb = [xt_pool.tile([P, S], BF, name=f"xk_{k}", tag="xt") for k in range(KB)]
        xv_sb = [xt_pool.tile([P, S], BF, name=f"xv_{k}", tag="xt") for k in range(KB)]
        b1_sb = wpool.tile([P, MF], F32, name="b1_sb", tag="b1_sb")
        b2_sb = wpool.tile([P, MF], F32, name="b2_sb", tag="b2_sb")
        b3_sb = wpool.tile([1, VW], BF, name="b3_sb", tag="b3_sb")
        ones_row = wpool.tile([1, P], BF, name="ones_row", tag="ones_row")
        nc.vector.memset(ones_row[:], 1.0)

        # two parallel HWDGE queues: sync carries the big x tensors in
        # criticality order, scalar carries all weights + biases
        for k in range(KB):
            nc.scalar.dma_start(w1_sb[k][:], w1[k * P:(k + 1) * P, :])
        for k in range(KB):
            nc.sync.dma_start(xq_sb[k][:], xq[k * P:(k + 1) * P, :])
        for k in range(KB):
            nc.scalar.dma_start(w2_sb[k][:], w2[k * P:(k + 1) * P, :])
        for k in range(KB):
            nc.scalar.dma_start(w3_sb[k][:], w3[k * P:(k + 1) * P, :])
        for k in range(KB):
            nc.sync.dma_start(xk_sb[k][:], xk[k * P:(k + 1) * P, :])
        for k in range(KB):
            nc.sync.dma_start(xv_sb[k][:], xv[k * P:(k + 1) * P, :])
        for k in range(MF):
            nc.scalar.dma_start(w4_sb[k][:], w4[k * P:(k + 1) * P, :])
        nc.scalar.dma_start(b1_sb[:], b1[:])
        nc.scalar.dma_start(b2_sb[:], b2[:])
        nc.scalar.dma_start(b3_sb[:], b3[:])

        # persistent activations
        qT = [persist.tile([P, S], BF, name=f"qT_{m}", tag=f"qT_{m}") for m in range(MF)]
        kT = [persist.tile([P, S], BF, name=f"kT_{m}", tag=f"kT_{m}") for m in range(MF)]
        v_sb = [persist.tile([P, VW], BF, name=f"v_{s}", tag=f"v_{s}") for s in range(SM)]
        outT = [persist.tile([P, S], BF, name=f"outT_{m}", tag=f"outT_{m}") for m in range(MF)]

        # ---- one q/k projection group: dst[m][:, 512-col slice] ----
        def proj_qk(name, x_sb, w_sb, b_sb, dst, m, h):
            csl = slice(h * 512, (h + 1) * 512)
            ps = psum.tile([P, 512], F32, name=f"pp_{name}_{m}_{h}", tag="sc", bufs=2)
            for k in range(KB):
                nc.tensor.matmul(
                    ps[:],
                    w_sb[k][:, m * P:(m + 1) * P],
                    x_sb[k][:, csl],
                    start=(k == 0),
                    stop=(k == KB - 1),
                )
            nc.vector.tensor_scalar_add(dst[m][:, csl], ps[:], b_sb[:, m:m + 1])

        # upfront: m=0 projections of q and k, k-major over 4 concurrent
        # [P,512] psum groups so each x tile is consumed as its DMA lands
        def proj_m0_kmajor(name, x_sb, w_sb, b_sb, dst):
            tags = ["sc", "sc", "pv", "pv"]
            pss = [
                psum.tile([P, 512], F32, name=f"pp0_{name}_{h}", tag=tags[h], bufs=2)
                for h in range(4)
            ]
            for k in range(KB):
                for h in range(4):
                    nc.tensor.matmul(
                        pss[h][:],
                        w_sb[k][:, 0:P],
                        x_sb[k][:, h * 512:(h + 1) * 512],
                        start=(k == 0),
                        stop=(k == KB - 1),
                    )
            for h in range(4):
                nc.vector.tensor_scalar_add(
                    dst[0][:, h * 512:(h + 1) * 512], pss[h][:], b_sb[:, 0:1]
                )

        proj_m0_kmajor("q", xq_sb, w1_sb, b1_sb, qT)
        # m=1 q projection here: dense PE work that only needs xq, covering
        # the wait for the xk DMA before the k projections can run
        for h in range(4):
            proj_qk("q1", xq_sb, w1_sb, b1_sb, qT, 1, h)
        proj_m0_kmajor("k", xk_sb, w2_sb, b2_sb, kT)

        # ---- filler generators (consumed inside windows at PE idle slots) ----
        def gen_vproj():
            for s in range(SM):
                ps = psum.tile([P, VW], F32, name=f"pv_{s}", tag="sc", bufs=2)
                for k in range(KB):
                    nc.tensor.matmul(
                        ps[:],
                        xv_sb[k][:, s * P:(s + 1) * P],
                        w3_sb[k][:],
                        start=(k == 0),
                        stop=False,
                    )
                nc.tensor.matmul(ps[:], ones_row[:], b3_sb[:], start=False, stop=True)
                nc.vector.tensor_copy(v_sb[s][:], ps[:])
                yield

        def gen_m1proj():
            for h in range(4):
                proj_qk("k1", xk_sb, w2_sb, b2_sb, kT, 1, h)
                yield

        def gen_w4(qts, alt_copy=False):
            for i, qt in enumerate(qts):
                ps = psum.tile([P, D], F32, name=f"po_{qt}", tag="sc", bufs=2)
                for oc in range(D // 512):
                    for m in range(MF):
                        nc.tensor.matmul(
                            ps[:, oc * 512:(oc + 1) * 512],
                            outT[m][:, qt * P:(qt + 1) * P],
                            w4_sb[m][:, oc * 512:(oc + 1) * 512],
                            start=(m == 0),
                            stop=(m == MF - 1),
                        )
                ob = stage.tile([P, D], BF, name=f"ob_{qt}", tag="ob")
                if alt_copy and i % 2 == 1:
                    nc.scalar.copy(ob[:], ps[:])
                else:
                    nc.vector.tensor_copy(ob[:], ps[:])
                nc.sync.dma_start(out[qt * P:(qt + 1) * P, :], ob[:])
                yield

        # ---- attention window: head-pair hp, query window qw.
        #      scores(kt) / PV(kt-2) interleave; filler consumed each step. ----
        def window(hp, qw, filler=None, fill_every=1, fill_start=0, drain=False,
                   lag=2):
            qsl = slice(qw * QW, (qw + 1) * QW)
            attn_t = [[None] * SM for _ in range(2)]
            pv_ps = [
                psum.tile([P, QW], F32, name=f"pvps_{hp}_{qw}_{h2}", tag="pv", bufs=2)
                for h2 in range(2)
            ]

            def emit_scores(kt):
                for h2 in range(2):
                    rsl = slice(h2 * DK, (h2 + 1) * DK)
                    ps = psum.tile([P, QW], F32, name=f"sc_{hp}_{qw}_{kt}_{h2}",
                                   tag="sc", bufs=2)
                    for half in range(2):
                        nc.tensor.matmul(
                            ps[:, half * 512:(half + 1) * 512],
                            kT[hp][rsl, kt * P:(kt + 1) * P],
                            qT[hp][rsl, qw * QW + half * 512: qw * QW + (half + 1) * 512],
                            start=True,
                            stop=True,
                        )
                    at = attn_pool.tile([P, QW], BF, name=f"at_{hp}_{qw}_{kt}_{h2}",
                                        tag="attnT", bufs=10)
                    nc.scalar.activation(
                        at[:], ps[:], mybir.ActivationFunctionType.Exp,
                        scale=1.0 / np.sqrt(DK),
                    )
                    attn_t[h2][kt] = at

            def emit_pv(kt, h2s=(0, 1)):
                for h2 in h2s:
                    h = hp * 2 + h2
                    vsl = slice(h * (DK + 1), h * (DK + 1) + DK + 1)
                    for half in range(2):
                        nc.tensor.matmul(
                            pv_ps[h2][0:DK + 1, half * 512:(half + 1) * 512],
                            v_sb[kt][:, vsl],
                            attn_t[h2][kt][:, half * 512:(half + 1) * 512],
                            start=(kt == 0),
                            stop=(kt == SM - 1),
                        )

            def norm(h2):
                # den copy -> fast reciprocal -> gpsimd partition broadcast
                # -> one DVE multiply into outT, in column halves
                for half in range(2):
                    hsl = slice(half * 512, (half + 1) * 512)
                    osl = slice(qw * QW + half * 512, qw * QW + (half + 1) * 512)
                    den = small.tile([1, 512], F32, name=f"den_{hp}_{qw}_{h2}_{half}",
                                     tag="den", bufs=3)
                    nc.vector.tensor_copy(den[:], pv_ps[h2][DK:DK + 1, hsl])
                    rec = small.tile([1, 512], F32, name=f"rec_{hp}_{qw}_{h2}_{half}",
                                     tag="rec", bufs=3)
                    nc.vector.reciprocal_approx_fast(rec[:], den[:])
                    bc = small.tile([DK, 512], F32, name=f"bc_{hp}_{qw}_{h2}_{half}",
                                    tag="bc", bufs=2)
                    nc.gpsimd.partition_broadcast(bc[:], rec[:])
                    nc.vector.tensor_mul(
                        outT[hp][h2 * DK:(h2 + 1) * DK, osl], pv_ps[h2][0:DK, hsl], bc[:]
                    )

            for kt in range(SM):
                emit_scores(kt)
                if (filler is not None and kt >= fill_start
                        and (kt - fill_start) % fill_every == 0):
                    next(filler, None)
                if kt >= lag:
                    emit_pv(kt - lag)
            if drain and filler is not None:
                for _ in filler:
                    pass
            # trailing PVs interleaved with norm per head so the DVE/gpsimd
            # norm chain of h2=0 overlaps the PE trailing PVs of h2=1
            for kt in range(SM - lag, SM):
                emit_pv(kt, h2s=(0,))
            norm(0)
            for kt in range(SM - lag, SM):
                emit_pv(kt, h2s=(1,))
            norm(1)

        window(0, 0, filler=gen_vproj(), fill_every=1, fill_start=2, drain=True, lag=4)
        window(0, 1, filler=gen_m1proj(), fill_every=2, fill_start=1)
        window(1, 0)
        window(1, 1, filler=gen_w4(range(SM // 2)), fill_every=2, fill_start=2,
               drain=True)
        for _ in gen_w4(range(SM // 2, SM), alt_copy=True):
            pass


_NC_CACHE = None


def _get_nc():
    global _NC_CACHE
    if _NC_CACHE is None:
        _NC_CACHE = _build_kernel()
    return _NC_CACHE


def _make_in_maps(query, key, value, W1, b1, W2, b2, W3, b3, W4, b4):
    in_maps = []
    for c in range(N_CORES):
        b, g = divmod(c, 4)
        gs = slice(g * F, (g + 1) * F)
        w3g = W3[gs, :].T.astype(np.float32)          # [D, F]
        w3i = np.zeros((D, VW), np.float32)
        b3g = b3[gs].astype(np.float32)
        b3i = np.zeros((VW,), np.float32)
        for h in range(H_CORE):
            w3i[:, h * (DK + 1): h * (DK + 1) + DK] = w3g[:, h * DK:(h + 1) * DK]
            b3i[h * (DK + 1): h * (DK + 1) + DK] = b3g[h * DK:(h + 1) * DK]
            b3i[h * (DK + 1) + DK] = 1.0
        in_maps.append({
            "xq_t": np.ascontiguousarray(query[b].T).astype(BF16),
            "xk_t": np.ascontiguousarray(key[b].T).astype(BF16),
            "xv_t": np.ascontiguousarray(value[b].T).astype(BF16),
            "w1t": np.ascontiguousarray(W1[gs, :].T).astype(BF16),
            "w2t": np.ascontiguousarray(W2[gs, :].T).astype(BF16),
            "w3i": np.ascontiguousarray(w3i).astype(BF16),
            "w4t": np.ascontiguousarray(W4[:, gs].T).astype(BF16),
            "b1c": np.ascontiguousarray(b1[gs].reshape(F // P, P).T).astype(np.float32),
            "b2c": np.ascontiguousarray(b2[gs].reshape(F // P, P).T).astype(np.float32),
            "b3i": b3i.reshape(1, VW).astype(BF16),
        })
    return in_maps


def kernel(query, key, value, W1, b1, W2, b2, W3, b3, W4, b4, _trace=False, _tmpdir=None):
    args = [np.asarray(a) for a in (query, key, value, W1, b1, W2, b2, W3, b3, W4, b4)]
    nc = _get_nc()
    in_maps = _make_in_maps(*args)
    res = run_bass_kernel_spmd(
        nc, in_maps, core_ids=list(range(N_CORES)),
        trace=_trace, tmpdir=_tmpdir,
    )
    b4_f = args[10].astype(np.float32)
    full = np.zeros((B, S, D), np.float32)
    for c in range(N_CORES):
        full[c // 4] += res.results[c]["out"]
    full += b4_f[None, None, :]
    kernel.last_results = res
    return full


# revision 23
# speedup vs baseline: 1.2514x; 1.0020x over previous
"""Multi-head attention (B=2, S=2048, D=1024, H=16, d_k=64) on 8 NeuronCores.

Sharding: 8 cores = 2 batches x 4 head-groups (4 heads each).
Core c handles batch b = c//4 and heads 4*(c%4) .. 4*(c%4)+4 (feature
slice of width F=256). Each core computes its partial output-projection
contribution [S, D]; the host sums the 4 head-group partials per batch
and adds b4 (the "all-reduce" of the row-sharded W4 projection).

Device dataflow works in a "transposed world" so every matmul operand
is in its natural PE layout (contraction on partitions), with zero
on-device transposes:
  qT = W1g @ x_q.T  [F, S]
  kT = W2g @ x_k.T  [F, S]
  v  = x_v @ W3i    [S, 260]  (260 = 4 heads x (64 v cols + 1 ones col);
                               the ones col comes from the bias matmul
                               with b3i[h*65+64] = 1, W3i zero there)
  scoresT_h = kT_h.T @ qT_h   [S_keys, S_q]  (K = d_k = 64)
  attnT = exp(scoresT / 8)    ACT, PSUM->SBUF bf16, no max subtraction
  pv = v_ext.T @ attnT        [65, S_q]; row 64 = softmax denominator
  outT_h = pv[0:64] * (1/pv[64])   (reciprocal_approx_fast + gpsimd
                                    partition_broadcast + one DVE mul)
  partial = outT.T @ W4g.T    [S, D]

All matmuls bf16 with f32 PSUM accumulation.

Schedule (built to keep the PE stream dense so the HAM clock gate stays
at 8/8, and to hide everything under the ACT exp roofline):
  - prioritized chunked DMA: w1 + xq first (512-col chunks) so the
    first projection matmuls issue ~5us in
  - upfront PE work: only the m=0 half of the q/k projections (enough
    for head-pair 0's windows)
  - window (0,0): scores/exp/PV with the v-projection interleaved as
    PE filler (one s-tile per kt step, sharing the "sc" psum tag)
  - window (0,1): m=1 q/k projection groups as filler
  - window (1,0): no filler
  - window (1,1): W4 output projection for qw=0 as filler
  - tail: W4 for qw=1
  PV lags scores by 2 key tiles so PV never waits on the exp of the
  same step (ACT latency hidden at any clock).  PSUM: scores 2x2 banks
  (tag sc, shared by all filler psum) + PV accumulators 2x2 banks.
"""

import numpy as np
import ml_dtypes

import concourse.bass as bass
import concourse.mybir as mybir
import concourse.tile as tile
from concourse import bacc
from concourse.bass_utils import run_bass_kernel_spmd

BF16 = ml_dtypes.bfloat16
F32 = mybir.dt.float32
BF = mybir.dt.bfloat16

B, S, D = 2, 2048, 1024
H_CORE = 4          # heads per core
DK = 64             # head dim
F = H_CORE * DK     # features per core = 256
P = 128             # partitions
KB = D // P         # k blocks in D contraction = 8
SM = S // P         # seq tiles of 128 = 16
QW = 1024           # query window width
NQW = S // QW       # query windows = 2
VW = H_CORE * (DK + 1)  # 260: v with interleaved ones columns
N_CORES = 8


def _build_kernel():
    nc = bacc.Bacc(
        "TRN2",
        target_bir_lowering=False,
        debug=False,
        enable_asserts=False,
        num_devices=N_CORES,
    )

    xq = nc.dram_tensor("xq_t", [D, S], BF, kind="ExternalInput").ap()
    xk = nc.dram_tensor("xk_t", [D, S], BF, kind="ExternalInput").ap()
    xv = nc.dram_tensor("xv_t", [D, S], BF, kind="ExternalInput").ap()
    w1 = nc.dram_tensor("w1t", [D, F], BF, kind="ExternalInput").ap()
    w2 = nc.dram_tensor("w2t", [D, F], BF, kind="ExternalInput").ap()
    w3 = nc.dram_tensor("w3i", [D, VW], BF, kind="ExternalInput").ap()
    w4 = nc.dram_tensor("w4t", [F, D], BF, kind="ExternalInput").ap()
    b1 = nc.dram_tensor("b1c", [P, F // P], F32, kind="ExternalInput").ap()
    b2 = nc.dram_tensor("b2c", [P, F // P], F32, kind="ExternalInput").ap()
    b3 = nc.dram_tensor("b3i", [1, VW], BF, kind="ExternalInput").ap()
    out = nc.dram_tensor("out", [S, D], BF, kind="ExternalOutput").ap()

    with tile.TileContext(nc) as tc:
        _body(tc, xq, xk, xv, w1, w2, w3, w4, b1, b2, b3, out)

    nc.compile()
    return nc


def _body(tc, xq, xk, xv, w1, w2, w3, w4, b1, b2, b3, out):
    nc = tc.nc
    MF = F // P  # m tiles for the F=256 feature dim = 2

    with (
        tc.tile_pool(name="wpool", bufs=1) as wpool,
        tc.tile_pool(name="xt", bufs=24) as xt_pool,
        tc.tile_pool(name="persist", bufs=1) as persist,
        tc.tile_pool(name="attn", bufs=10) as attn_pool,
        tc.tile_pool(name="small", bufs=3) as small,
        tc.tile_pool(name="stage", bufs=2) as stage,
        tc.tile_pool(name="psum", bufs=1, space="PSUM") as psum,
    ):
        # ---- SBUF tiles; DMA issued in priority order ----
        w1_sb = [wpool.tile([P, F], BF, name=f"w1_{k}", tag=f"w1_{k}") for k in range(KB)]
        w2_sb = [wpool.tile([P, F], BF, name=f"w2_{k}", tag=f"w2_{k}") for k in range(KB)]
        w3_sb = [wpool.tile([P, VW], BF, name=f"w3_{k}", tag=f"w3_{k}") for k in range(KB)]
        w4_sb = [wpool.tile([P, D], BF, name=f"w4_{k}", tag=f"w4_{k}") for k in range(MF)]
        xq_sb = [xt_pool.tile([P, S], BF, name=f"xq_{k}", tag="xt") for k in range(KB)]
        xk_sb = [xt_pool.tile([P, S], BF, name=f"xk_{k}", tag="xt") for k in range(KB)]
        xv_sb = [xt_pool.tile([P, S], BF, name=f"xv_{k}", tag="xt") for k in range(KB)]
        b1_sb = wpool.tile([P, MF], F32, name="b1_sb", tag="b1_sb")
        b2_sb = wpool.tile([P, MF], F32, name="b2_sb", tag="b2_sb")
        b3_sb = wpool.tile([1, VW], BF, name="b3_sb", tag="b3_sb")
        ones_row = wpool.tile([1, P], BF, name="ones_row", tag="ones_row")
        nc.vector.memset(ones_row[:], 1.0)

        # two parallel HWDGE queues: sync carries the big x tensors in
        # criticality order, scalar carries all weights + biases
        for k in range(KB):
            nc.scalar.dma_start(w1_sb[k][:], w1[k * P:(k + 1) * P, :])
        for k in range(KB):
            nc.sync.dma_start(xq_sb[k][:], xq[k * P:(k + 1) * P, :])
        for k in range(KB):
            nc.scalar.dma_start(w2_sb[k][:], w2[k * P:(k + 1) * P, :])
        for k in range(KB):
            nc.scalar.dma_start(w3_sb[k][:], w3[k * P:(k + 1) * P, :])
        for k in range(KB):
            nc.sync.dma_start(xk_sb[k][:], xk[k * P:(k + 1) * P, :])
        for k in range(KB):
            nc.sync.dma_start(xv_sb[k][:], xv[k * P:(k + 1) * P, :])
        for k in range(MF):
            nc.scalar.dma_start(w4_sb[k][:], w4[k * P:(k + 1) * P, :])
        nc.scalar.dma_start(b1_sb[:], b1[:])
        nc.scalar.dma_start(b2_sb[:], b2[:])
        nc.scalar.dma_start(b3_sb[:], b3[:])

        # persistent activations
        qT = [persist.tile([P, S], BF, name=f"qT_{m}", tag=f"qT_{m}") for m in range(MF)]
        kT = [persist.tile([P, S], BF, name=f"kT_{m}", tag=f"kT_{m}") for m in range(MF)]
        v_sb = [persist.tile([P, VW], BF, name=f"v_{s}", tag=f"v_{s}") for s in range(SM)]
        outT = [persist.tile([P, S], BF, name=f"outT_{m}", tag=f"outT_{m}") for m in range(MF)]

        # ---- one q/k projection group: dst[m][:, 512-col slice] ----
        def proj_qk(name, x_sb, w_sb, b_sb, dst, m, h):
            csl = slice(h * 512, (h + 1) * 512)
            ps = psum.tile([P, 512], F32, name=f"pp_{name}_{m}_{h}", tag="sc", bufs=2)
            for k in range(KB):
                nc.tensor.matmul(
                    ps[:],
                    w_sb[k][:, m * P:(m + 1) * P],
                    x_sb[k][:, csl],
                    start=(k == 0),
                    stop=(k == KB - 1),
                )
            nc.vector.tensor_scalar_add(dst[m][:, csl], ps[:], b_sb[:, m:m + 1])

        # upfront: m=0 projections of q and k, k-major over 4 concurrent
        # [P,512] psum groups so each x tile is consumed as its DMA lands
        def proj_m0_kmajor(name, x_sb, w_sb, b_sb, dst):
            tags = ["sc", "sc", "pv", "pv"]
            pss = [
                psum.tile([P, 512], F32, name=f"pp0_{name}_{h}", tag=tags[h], bufs=2)
                for h in range(4)
            ]
            for k in range(KB):
                for h in range(4):
                    nc.tensor.matmul(
                        pss[h][:],
                        w_sb[k][:, 0:P],
                        x_sb[k][:, h * 512:(h + 1) * 512],
                        start=(k == 0),
                        stop=(k == KB - 1),
                    )
            for h in range(4):
                nc.vector.tensor_scalar_add(
                    dst[0][:, h * 512:(h + 1) * 512], pss[h][:], b_sb[:, 0:1]
                )

        proj_m0_kmajor("q", xq_sb, w1_sb, b1_sb, qT)
        # m=1 q projection here: dense PE work that only needs xq, covering
        # the wait for the xk DMA before the k projections can run
        for h in range(4):
            proj_qk("q1", xq_sb, w1_sb, b1_sb, qT, 1, h)
        proj_m0_kmajor("k", xk_sb, w2_sb, b2_sb, kT)

        # ---- filler generators (consumed inside windows at PE idle slots) ----
        def gen_vproj():
            for s in range(SM):
                ps = psum.tile([P, VW], F32, name=f"pv_{s}", tag="sc", bufs=2)
                for k in range(KB):
                    nc.tensor.matmul(
                        ps[:],
                        xv_sb[k][:, s * P:(s + 1) * P],
                        w3_sb[k][:],
                        start=(k == 0),
                        stop=False,
                    )
                nc.tensor.matmul(ps[:], ones_row[:], b3_sb[:], start=False, stop=True)
                nc.vector.tensor_copy(v_sb[s][:], ps[:])
                yield

        def gen_m1proj():
            for h in range(4):
                proj_qk("k1", xk_sb, w2_sb, b2_sb, kT, 1, h)
                yield

        def gen_w4(qts, alt_copy=False):
            for i, qt in enumerate(qts):
                ps = psum.tile([P, D], F32, name=f"po_{qt}", tag="sc", bufs=2)
                for oc in range(D // 512):
                    for m in range(MF):
                        nc.tensor.matmul(
                            ps[:, oc * 512:(oc + 1) * 512],
                            outT[m][:, qt * P:(qt + 1) * P],
                            w4_sb[m][:, oc * 512:(oc + 1) * 512],
                            start=(m == 0),
                            stop=(m == MF - 1),
                        )
                ob = stage.tile([P, D], BF, name=f"ob_{qt}", tag="ob")
                if alt_copy and i % 2 == 1:
                    nc.scalar.copy(ob[:], ps[:])
                else:
                    nc.vector.tensor_copy(ob[:], ps[:])
                nc.sync.dma_start(out[qt * P:(qt + 1) * P, :], ob[:])
                yield

        # ---- attention window: head-pair hp, query window qw.
        #      scores(kt) / PV(kt-2) interleave; filler consumed each step. ----
        def window(hp, qw, filler=None, fill_every=1, fill_start=0, drain=False,
                   lag=2):
            qsl = slice(qw * QW, (qw + 1) * QW)
            attn_t = [[None] * SM for _ in range(2)]
            pv_ps = [
                psum.tile([P, QW], F32, name=f"pvps_{hp}_{qw}_{h2}", tag="pv", bufs=2)
                for h2 in range(2)
            ]

            def emit_scores(kt):
                for h2 in range(2):
                    rsl = slice(h2 * DK, (h2 + 1) * DK)
                    ps = psum.tile([P, QW], F32, name=f"sc_{hp}_{qw}_{kt}_{h2}",
                                   tag="sc", bufs=2)
                    for half in range(2):
                        nc.tensor.matmul(
                            ps[:, half * 512:(half + 1) * 512],
                            kT[hp][rsl, kt * P:(kt + 1) * P],
                            qT[hp][rsl, qw * QW + half * 512: qw * QW + (half + 1) * 512],
                            start=True,
                            stop=True,
                        )
                    at = attn_pool.tile([P, QW], BF, name=f"at_{hp}_{qw}_{kt}_{h2}",
                                        tag="attnT", bufs=10)
                    nc.scalar.activation(
                        at[:], ps[:], mybir.ActivationFunctionType.Exp,
                        scale=1.0 / np.sqrt(DK),
                    )
                    attn_t[h2][kt] = at

            def emit_pv(kt, h2s=(0, 1)):
                for h2 in h2s:
                    h = hp * 2 + h2
                    vsl = slice(h * (DK + 1), h * (DK + 1) + DK + 1)
                    for half in range(2):
                        nc.tensor.matmul(
                            pv_ps[h2][0:DK + 1, half * 512:(half + 1) * 512],
                            v_sb[kt][:, vsl],
                            attn_t[h2][kt][:, half * 512:(half + 1) * 512],
                            start=(kt == 0),
                            stop=(kt == SM - 1),
                        )

            def norm_all():
                # batched phases: all den copies + reciprocals (DVE), then
                # all broadcasts (gpsimd), then all multiplies (DVE) so the
                # two engines pipeline instead of serializing per chunk
                recs, bcs = {}, {}
                for h2 in range(2):
                    for half in range(2):
                        hsl = slice(half * 512, (half + 1) * 512)
                        den = small.tile([1, 512], F32,
                                         name=f"den_{hp}_{qw}_{h2}_{half}",
                                         tag="den", bufs=3)
                        nc.vector.tensor_copy(den[:], pv_ps[h2][DK:DK + 1, hsl])
                        rec = small.tile([1, 512], F32,
                                         name=f"rec_{hp}_{qw}_{h2}_{half}",
                                         tag="rec", bufs=3)
                        nc.vector.reciprocal_approx_fast(rec[:], den[:])
                        recs[h2, half] = rec
                for h2 in range(2):
                    for half in range(2):
                        bc = small.tile([DK, 512], F32,
                                        name=f"bc_{hp}_{qw}_{h2}_{half}",
                                        tag="bc", bufs=4)
                        nc.gpsimd.partition_broadcast(bc[:], recs[h2, half][:])
                        bcs[h2, half] = bc
                for h2 in range(2):
                    for half in range(2):
                        hsl = slice(half * 512, (half + 1) * 512)
                        osl = slice(qw * QW + half * 512, qw * QW + (half + 1) * 512)
                        nc.vector.tensor_mul(
                            outT[hp][h2 * DK:(h2 + 1) * DK, osl],
                            pv_ps[h2][0:DK, hsl], bcs[h2, half][:]
                        )

            for kt in range(SM):
                emit_scores(kt)
                if (filler is not None and kt >= fill_start
                        and (kt - fill_start) % fill_every == 0):
                    next(filler, None)
                if kt >= lag:
                    emit_pv(kt - lag)
            if drain and filler is not None:
                for _ in filler:
                    pass
            for kt in range(SM - lag, SM):
                emit_pv(kt)
            norm_all()

        window(0, 0, filler=gen_vproj(), fill_every=1, fill_start=2, drain=True, lag=4)
        window(0, 1, filler=gen_m1proj(), fill_every=2, fill_start=1)
        window(1, 0)
        window(1, 1, filler=gen_w4(range(SM // 2)), fill_every=2, fill_start=2,
               drain=True)
        for _ in gen_w4(range(SM // 2, SM), alt_copy=True):
            pass


_NC_CACHE = None


def _get_nc():
    global _NC_CACHE
    if _NC_CACHE is None:
        _NC_CACHE = _build_kernel()
    return _NC_CACHE


def _make_in_maps(query, key, value, W1, b1, W2, b2, W3, b3, W4, b4):
    in_maps = []
    for c in range(N_CORES):
        b, g = divmod(c, 4)
        gs = slice(g * F, (g + 1) * F)
        w3g = W3[gs, :].T.astype(np.float32)          # [D, F]
        w3i = np.zeros((D, VW), np.float32)
        b3g = b3[gs].astype(np.float32)
        b3i = np.zeros((VW,), np.float32)
        for h in range(H_CORE):
            w3i[:, h * (DK + 1): h * (DK + 1) + DK] = w3g[:, h * DK:(h + 1) * DK]
            b3i[h * (DK + 1): h * (DK + 1) + DK] = b3g[h * DK:(h + 1) * DK]
            b3i[h * (DK + 1) + DK] = 1.0
        in_maps.append({
            "xq_t": np.ascontiguousarray(query[b].T).astype(BF16),
            "xk_t": np.ascontiguousarray(key[b].T).astype(BF16),
            "xv_t": np.ascontiguousarray(value[b].T).astype(BF16),
            "w1t": np.ascontiguousarray(W1[gs, :].T).astype(BF16),
            "w2t": np.ascontiguousarray(W2[gs, :].T).astype(BF16),
            "w3i": np.ascontiguousarray(w3i).astype(BF16),
            "w4t": np.ascontiguousarray(W4[:, gs].T).astype(BF16),
            "b1c": np.ascontiguousarray(b1[gs].reshape(F // P, P).T).astype(np.float32),
            "b2c": np.ascontiguousarray(b2[gs].reshape(F // P, P).T).astype(np.float32),
            "b3i": b3i.reshape(1, VW).astype(BF16),
        })
    return in_maps


def kernel(query, key, value, W1, b1, W2, b2, W3, b3, W4, b4, _trace=False, _tmpdir=None):
    args = [np.asarray(a) for a in (query, key, value, W1, b1, W2, b2, W3, b3, W4, b4)]
    nc = _get_nc()
    in_maps = _make_in_maps(*args)
    res = run_bass_kernel_spmd(
        nc, in_maps, core_ids=list(range(N_CORES)),
        trace=_trace, tmpdir=_tmpdir,
    )
    b4_f = args[10].astype(np.float32)
    full = np.zeros((B, S, D), np.float32)
    for c in range(N_CORES):
        full[c // 4] += res.results[c]["out"]
    full += b4_f[None, None, :]
    kernel.last_results = res
    return full
